# revision 47
# baseline (speedup 1.0000x reference)
"""Longformer self-attention Trainium2 kernel (8-core SPMD).

Sharding: core c handles batch b = c//4 and heads [3*(c%4), 3*(c%4)+3).
Each core receives pre-sliced/augmented inputs and computes [4096, 192]
(its 3 heads' output dims); the host reassembles [2, 4096, 768].

Device-side math per core (heads h in 0..3, all layouts chosen so no
on-device transposes are needed):
  - xT [768, 4096] = hidden[b].T; q-scale folded into Wq/Wqg on host.
  - q/k projections packed into one [768, 384] weight (column order
    q0,q1,k0,k1,q2,k2) so PSUM tiles are full 128 rows and evacuate
    with full-lane DVE ops; heads 0/1 of each projection live stacked
    in one [128, S] SBUF tile (head h at partition base 64*(h%2), so
    every per-head matmul has lhsT/rhs at matching partition bases).
  - kg/vg (only consumed by the 16 global-token rows, whose softmax
    averages over all 4096 keys) are computed in fp8e4m3 with the
    DoubleRow perf mode (2 contraction chunks per instruction = 2x
    fewer PE instructions). Weights are pre-scaled by 64 on the host to
    sit in the e4m3 normal range; the 1/64 descale is folded into the
    bias-add evacuation.
  - Band scores computed transposed: sT[kpos, q]. Each 128-query half
    consumes only 5 kpos chunks, so the half-specific edge chunks
    (c=0 -> half 0, c=5 -> half 1) are computed at N=128 and share
    score slot 0 of a [128, 5, 256] PSUM tile.
  - exp() without max subtraction (logits are O(0.3)); band-validity
    and global-exclusion masks are [128, 128] triangles applied
    multiplicatively after the exp, split across DVE and gpsimd.
  - Global columns (sel): the three heads' [16, S] score tiles are
    packed at partition offsets {0, 32, 64} of one [96, S] tensor via
    matmul tile positioning so the exp runs on 96 lanes instead of 16.
  - PV: attn[q, 0:64] and the softmax denominator (ones column of v)
    come out of one accumulated PSUM [128, 65]; normalize = reciprocal
    + mul.
  - Band block t only needs projection s-tiles <= ceil(t/2), so blocks
    2s-1 and 2s are interleaved right after s-tile s: the band's
    scalar/DVE-heavy pipeline fills the projection phase's DMA/evac
    stalls and smooths tensor-engine utilization (the HW power governor
    halves the PE clock when utilization stays pinned near 100%).
  - Global-token rows (0..15) use the qg/kg/vg projections with the
    same transposed-score trick; block 0 (whose rows 0..15 they
    overwrite) runs last.
"""

import sys

sys.path.insert(0, "/opt/trn_rl_repo")

import numpy as np
import ml_dtypes

B, S, Dm, H, WIN, G, HD = 2, 4096, 768, 12, 256, 16, 64
HPC = 3            # heads per core
NCORES = 8
DPC = HPC * HD     # 192 output dims per core
NB = S // WIN      # 16 query blocks
NKC = S // 128     # 32 kpos chunks of 128
SCALE = 1.0 / 8.0  # 1/sqrt(64)
FP8S = 64.0        # fp8 weight pre-scale (host) / descale (evacuation)
QS8 = 512.0        # fp8 pre-scale for q columns (Wq*SCALE has std 0.0025)
KS8 = 64.0         # fp8 pre-scale for k columns

_CACHE = {}


def _mask_classes():
    """Multiplicative {0,1} masks in transposed-score orientation
    [kpos_local p, q_local r (within a 128-query half)], applied to
    exp(scores). Keep (1.0) iff the slot is band-valid and not a global
    key; masked slots contribute exactly 0 to the reference softmax
    (exp(-inf) / exp(x - 10000) both underflow to 0).

    Each 128-query half i (q = 128i + r) consumes kpos chunks
    j = i-2 .. i+2. Only the edge chunks need masks: j = i-2 keeps
    p >= r (lower triangle), j = i+2 keeps p <= r; chunk j = 0
    additionally excludes the global keys (p >= G). Interior chunks are
    fully valid. Returns {name: [128, 128] mask}, plus a per-(t, c)
    application list [(name, half)] verified against the reference
    condition.
    """
    p = np.arange(128)[:, None]
    r = np.arange(128)[None, :]
    classes = {
        "lowT": (p >= r).astype(np.float32),
        "upT": (p <= r).astype(np.float32),
        "lowTg16": ((p >= r) & (p >= G)).astype(np.float32),
        "g16": (p >= G).astype(np.float32) * np.ones((128, 128), np.float32),
    }

    def ref_keep(t, c, half):
        # reference validity of chunk c's slots for query half (t, half)
        kpos = (2 * t - 2 + c) * 128 + p
        i = 256 * t + 128 * half + r
        return (np.abs(kpos - i) <= WIN) & (kpos >= 0) & (kpos < S) & (kpos >= G)

    # application list per (t, c): [(class_name or None, half), ...]
    apply = {}
    for t in range(NB):
        cl, ch = _chunk_range(t)
        for c in range(cl, ch):
            j = 2 * t - 2 + c
            ents = []
            for half in range(2):
                i = 2 * t + half
                if not (i - 2 <= j <= i + 2):
                    continue  # this half never consumes chunk c
                if j == i - 2:
                    nm = "lowTg16" if j == 0 else "lowT"
                elif j == i + 2:
                    nm = "upT"
                elif j == 0:
                    nm = "g16"
                else:
                    nm = None
                if nm is not None:
                    assert np.array_equal(
                        classes[nm].astype(bool), ref_keep(t, c, half)
                    ), (t, c, half, nm)
                else:
                    assert np.all(ref_keep(t, c, half)), (t, c, half)
                ents.append((nm, half))
            apply[(t, c)] = ents
    return classes, apply


def _chunk_range(t):
    if t == 0:
        return 2, 6
    if t == NB - 1:
        return 0, 4
    return 0, 6


def _patch_drain_and_barrier():
    """The walrus build in this container rejects >1 sync-wait on the CTRL
    (Drain) instruction that TileContext emits at exit ("Too many sync wait
    commands"). Split the waits: keep one on the drain, emit the rest as
    explicit single-sem wait_ge instructions on the sync engine before the
    barrier. Semantics preserved: all sems still quiesce before the
    sem-clear + barrier."""
    import concourse.tile as tile
    from concourse import mybir
    from concourse.vector_clock import ScopedClock

    if getattr(tile.TileContext, "_ant_drain_patch", False):
        return

    def _drain_and_barrier(self, tick_clock, wait_clock):
        nc = self.nc
        drain_inst = nc.sync.drain()
        wait_clock.add_sem_waits(
            drain_inst.ins, ScopedClock({None: tick_clock.global_clock})
        )
        si = drain_inst.ins.sync_info
        waits = list(si.on_wait) if si is not None else []
        if len(waits) > 1:
            drain_inst.ins.sync_info = mybir.SyncInfo(
                on_wait=[waits[0]], on_update=list(si.on_update)
            )
            allocated = self.sems.allocated()
            by_name = {}
            for key, sem in allocated.items():
                by_name[str(key)] = sem
                nm = getattr(sem, "name", None)
                if nm is not None:
                    by_name[str(nm)] = sem
            for w in waits[1:]:
                sem = by_name[w.ant_name]
                nc.sync.wait_ge(sem, w.wait_value)
        nc.all_engine_barrier()
        assert self.sems is not None
        popped = nc._tile_sem_poison_stack.pop()
        assert popped is self._sem_poison
        nc.clear_and_free_semaphores(list(self.sems.allocated().values()))
        nc.all_engine_barrier()

    tile.TileContext._drain_and_barrier = _drain_and_barrier
    tile.TileContext._ant_drain_patch = True


def _build_program():
    import concourse.bass as bass
    import concourse.tile as tile
    from concourse import bacc, mybir

    _patch_drain_and_barrier()

    f32 = mybir.dt.float32
    bf16 = mybir.dt.bfloat16
    fp8 = mybir.dt.float8e4
    AF = mybir.ActivationFunctionType
    ALU = mybir.AluOpType
    DR = mybir.MatmulPerfMode.DoubleRow

    # Bacc (not plain Bass): its compile() pipeline runs
    # generate_event_semaphores, which splits multi-sem waits — this
    # walrus build allows at most one sync wait per instruction.
    nc = bacc.Bacc(None)

    xT = nc.dram_tensor("xT", [Dm, S], bf16, kind="ExternalInput")
    x8T = nc.dram_tensor("x8T", [Dm, S], fp8, kind="ExternalInput")
    # column order q0,q1,k0,k1,q2,k2 (64 cols each; q cols pre-scaled)
    Wqk = nc.dram_tensor("Wqk", [Dm, 2 * DPC], bf16, kind="ExternalInput")
    W8kg = nc.dram_tensor("W8kg", [Dm, DPC], fp8, kind="ExternalInput")
    Wv = nc.dram_tensor("Wv", [Dm, DPC], bf16, kind="ExternalInput")
    W8vg = nc.dram_tensor("W8vg", [Dm, DPC], fp8, kind="ExternalInput")
    Wqg = nc.dram_tensor("Wqg", [Dm, DPC], bf16, kind="ExternalInput")
    # stacked bias columns: col layout matches the packed PSUM tiles
    b_qk = nc.dram_tensor("b_qk", [128, 3], f32, kind="ExternalInput")
    b_kg = nc.dram_tensor("b_kg", [128, 2], f32, kind="ExternalInput")
    b_qg = nc.dram_tensor("b_qg", [128, 2], f32, kind="ExternalInput")
    # broadcast v/vg biases: [128 partitions, head, 64]
    b_v = nc.dram_tensor("b_v", [128, HPC, HD], f32, kind="ExternalInput")
    b_vg = nc.dram_tensor("b_vg", [128, HPC, HD], f32, kind="ExternalInput")
    out_d = nc.dram_tensor("out", [S, DPC], f32, kind="ExternalOutput")

    classes, mask_apply = _mask_classes()
    mask_names = list(classes.keys())
    mask_np = np.stack([classes[k] for k in mask_names], axis=1)  # [128, 4, 128]
    masks_d = nc.inline_tensor(mask_np.astype(ml_dtypes.bfloat16), name="masks")
    midx = {k: i for i, k in enumerate(mask_names)}

    from contextlib import ExitStack

    with tile.TileContext(nc) as tc, ExitStack() as ctx:
        const = ctx.enter_context(tc.tile_pool(name="const", bufs=1))
        ph = ctx.enter_context(tc.tile_pool(name="ph", bufs=1))
        xpool = ctx.enter_context(tc.tile_pool(name="xpool", bufs=14))
        x8pool = ctx.enter_context(tc.tile_pool(name="x8pool", bufs=3))
        bx = ctx.enter_context(tc.tile_pool(name="bx", bufs=4))
        sbS = ctx.enter_context(tc.tile_pool(name="sbS", bufs=6))
        psA = ctx.enter_context(tc.tile_pool(name="psA", bufs=2, space="PSUM"))
        psB = ctx.enter_context(tc.tile_pool(name="psB", bufs=2, space="PSUM"))

        # issue the first projection group's operands first (Wqk chunk 0,
        # x chunk 0) so the PE starts within ~1us of kernel entry
        wqk = const.tile([128, 6, 2 * DPC], bf16, tag="wqk", name="wqk")
        nc.sync.dma_start(out=wqk[:, 0, :], in_=Wqk[0:128, :])
        nc.sync.dma_start(out=wqk[:, 1, :], in_=Wqk[128:256, :])

        def x_tiles(ssl, splits):
            # bf16 x chunks as len(splits) tiles; returns accessor kc -> AP
            tiles, offs = [], []
            k0 = 0
            for n in splits:
                t = xpool.tile([128, n, 512], bf16, tag="xt", name="xt")
                nc.sync.dma_start(
                    out=t,
                    in_=xT[128 * k0 : 128 * (k0 + n), ssl].rearrange(
                        "(c p) s -> p c s", p=128
                    ),
                )
                tiles.append(t)
                offs.append(k0)
                k0 += n
            def xtc(kc, cs=slice(0, 512)):
                for t, o, n in zip(tiles, offs, splits):
                    if o <= kc < o + n:
                        return t[:, kc - o, cs]
                raise KeyError(kc)
            return xtc

        def x8_tile(ssl):
            t8 = x8pool.tile([128, 6, 512], fp8, tag="xt8", name="xt8")
            nc.sync.dma_start(
                out=t8, in_=x8T[:, ssl].rearrange("(c p) s -> p c s", p=128)
            )
            return t8

        xt0 = x_tiles(slice(0, 512), (1, 1, 2, 2))
        nc.sync.dma_start(
            out=wqk[:, 2:6, :],
            in_=Wqk[256:768, :].rearrange("(c p) d -> p c d", p=128),
        )
        xt80 = x8_tile(slice(0, 512))

        # ---- remaining constants to SBUF ----
        w6 = {}
        for nm, dram, width, dt in (
            ("kg", W8kg, DPC, fp8),
            ("v", Wv, DPC, bf16),
            ("vg", W8vg, DPC, fp8),
            ("qg", Wqg, DPC, bf16),
        ):
            w6[nm] = const.tile([128, 6, width], dt, tag=f"w6{nm}", name=f"w6{nm}")
            nc.sync.dma_start(
                out=w6[nm], in_=dram[:, :].rearrange("(c p) d -> p c d", p=128)
            )
        bias = {}
        for nm, dram, w in (("qk", b_qk, 3), ("kg", b_kg, 2), ("qg", b_qg, 2)):
            bias[nm] = const.tile([128, w], f32, tag=f"b{nm}", name=f"b{nm}")
            nc.sync.dma_start(out=bias[nm], in_=dram[:])
        bv_sb = const.tile([128, HPC, HD], f32, tag="bv", name="bv_sb")
        nc.sync.dma_start(out=bv_sb, in_=b_v[:])
        bvg_sb = const.tile([128, HPC, HD], f32, tag="bvg", name="bvg_sb")
        nc.sync.dma_start(out=bvg_sb, in_=b_vg[:])
        masks_sb = const.tile([128, 4, 128], bf16, tag="masks", name="masks_sb")
        nc.sync.dma_start(out=masks_sb, in_=masks_d[:])

        # ---- persistent per-head tensors (heads 0/1 stacked per tile) ----
        P0 = ph.tile([128, S], bf16, tag="P0", name="P0")   # [q0; q1]
        P1 = ph.tile([128, S], bf16, tag="P1", name="P1")   # [k0; k1]
        q2 = ph.tile([64, S], bf16, tag="q2", name="q2")
        k2 = ph.tile([64, S], bf16, tag="k2", name="k2")
        KG01 = ph.tile([128, S], bf16, tag="KG01", name="KG01")
        kg2 = ph.tile([64, S], bf16, tag="kg2", name="kg2")
        QG01 = ph.tile([128, G], bf16, tag="QG01", name="QG01")
        qg2 = ph.tile([64, G], bf16, tag="qg2", name="qg2")

        def qTh(h, cs):
            return P0[64 * h : 64 * h + 64, cs] if h < 2 else q2[:, cs]

        def kTh(h, cs):
            return P1[64 * h : 64 * h + 64, cs] if h < 2 else k2[:, cs]

        def kgh(h, cs):
            return KG01[64 * h : 64 * h + 64, cs] if h < 2 else kg2[:, cs]

        def qgh(h):
            return QG01[64 * h : 64 * h + 64, :] if h < 2 else qg2[:, :]

        # v/vg interleaved with ones column: [:, chunk, 2h+0, :] = v head h,
        # [:, chunk, 2h+1, :] = vg head h ([:, :, :, 64] = 1.0)
        vall = ph.tile([128, NKC, 2 * HPC, HD + 1], bf16, tag="vall", name="vall")
        nc.vector.memset(vall[:, :, :, HD : HD + 1], 1.0)
        # three heads' global-column exp'd scores packed at partition
        # offsets {0, 32, 64}: rows 32h..32h+16 = head h's [16, S]
        selexp3 = ph.tile([96, S], bf16, tag="selexp3", name="selexp3")
        # v-global rows replicated at the same offsets for the PV matmul
        vg3 = ph.tile([96, HD + 1], bf16, tag="vg3", name="vg3")
        eg = [
            ph.tile([128, NKC, G], bf16, tag=f"eg{h}", name=f"eg{h}")
            for h in range(HPC)
        ]
        outg = [ph.tile([G, HD], f32, tag=f"outg{h}", name=f"outg{h}") for h in range(HPC)]

        def mm(out, lhsT, rhs, start, stop):
            nc.tensor.matmul(out, lhsT, rhs, start=start, stop=stop)

        AFexp = AF.Exp

        def vall_slot_ap(ci, par, width=HD):
            # [128, h, d] AP over vall slots (par=0: v slots 0/2/4;
            # par=1: vg slots 1/3/5) of kpos chunk ci
            return bass.AP(
                tensor=vall.tensor,
                offset=vall.offset + (ci * 2 * HPC + par) * (HD + 1),
                ap=[vall.ap[0], [2 * (HD + 1), HPC], [1, width]],
            )

        # ---- projection s-tile body ----
        def proj_stile(st):
            ssl = slice(512 * st, 512 * (st + 1))
            if st == 0:
                xt, xt8 = xt0, xt80
            else:
                xt8 = x8_tile(ssl)
                xt = x_tiles(ssl, (3, 3))

            # q/k packed: transposed layout, W stationary, 3 full PSUM
            # tiles [q0;q1], [k0;k1], [q2;k2]
            for dc in range(3):
                d0 = 128 * dc
                ps = psB.tile([128, 512], f32, tag="small", name="psqk")
                for kc in range(6):
                    mm(ps, wqk[:, kc, d0 : d0 + 128], xt(kc), kc == 0, kc == 5)
                if dc == 0:
                    nc.vector.tensor_scalar_add(P0[:, ssl], ps, bias["qk"][:, 0:1])
                elif dc == 1:
                    nc.vector.tensor_scalar_add(P1[:, ssl], ps, bias["qk"][:, 1:2])
                else:
                    nc.vector.tensor_scalar_add(
                        q2[:, ssl], ps[0:64, :], bias["qk"][0:64, 2:3]
                    )
                    nc.vector.tensor_scalar_add(
                        k2[:, ssl], ps[64:128, :], bias["qk"][64:128, 2:3]
                    )

            # kg: fp8 DoubleRow, transposed layout, W stationary
            for ti, (d0, d1) in enumerate(((0, 128), (128, 192))):
                ps = psB.tile([d1 - d0, 512], f32, tag="small", name="pskg")
                for p in range(3):
                    nc.tensor.matmul(
                        ps,
                        w6["kg"][:, 2 * p : 2 * p + 2, d0:d1],
                        xt8[:, 2 * p : 2 * p + 2, :],
                        start=(p == 0),
                        stop=(p == 2),
                        perf_mode=DR,
                    )
                dst = KG01[:, ssl] if ti == 0 else kg2[:, ssl]
                nc.vector.tensor_scalar(
                    dst,
                    ps,
                    1.0 / FP8S,
                    bias["kg"][0 : d1 - d0, ti : ti + 1],
                    ALU.mult,
                    ALU.add,
                )

            # v: natural layout, xT chunks stationary (bf16)
            for sc in range(4):
                ci = 4 * st + sc
                msl = slice(128 * sc, 128 * (sc + 1))
                psv = psB.tile([128, DPC], f32, tag="small", name="psv")
                for kc in range(6):
                    mm(psv, xt(kc, msl), w6["v"][:, kc, :], kc == 0, kc == 5)
                nc.vector.tensor_add(
                    vall_slot_ap(ci, 0),
                    psv[:, :].rearrange("p (h d) -> p h d", h=HPC),
                    bv_sb,
                )

                # vg: natural layout, fp8 DoubleRow, xT chunks stationary
                psg = psB.tile([128, DPC], f32, tag="small", name="psvg")
                for p in range(3):
                    nc.tensor.matmul(
                        psg,
                        xt8[:, 2 * p : 2 * p + 2, msl],
                        w6["vg"][:, 2 * p : 2 * p + 2, :],
                        start=(p == 0),
                        stop=(p == 2),
                        perf_mode=DR,
                    )
                nc.vector.scalar_tensor_tensor(
                    vall_slot_ap(ci, 1),
                    psg[:, :].rearrange("p (h d) -> p h d", h=HPC),
                    1.0 / FP8S,
                    bvg_sb,
                    ALU.mult,
                    ALU.add,
                )

            # global columns for this s-tile: sel = q . k[:G], all heads
            # packed into one [96, 512] PSUM tile so the exp uses 96 lanes
            sps = psB.tile([96, 512], f32, tag="small", name="sps")
            for h in range(HPC):
                mm(
                    sps[32 * h : 32 * h + G, :],
                    kTh(h, slice(0, G)),
                    qTh(h, ssl),
                    True,
                    True,
                )
            nc.scalar.activation(out=selexp3[:, ssl], in_=sps, func=AFexp)

            if st == 0:
                # qg: heads 0/1 into one [128, G] PSUM tile, head 2 separate
                psq = psB.tile([128, G], f32, tag="small", name="psqg")
                for mq in range(2):
                    for kc in range(6):
                        mm(
                            psq[64 * mq : 64 * mq + 64, :],
                            w6["qg"][:, kc, 64 * mq : 64 * mq + 64],
                            xt(kc, slice(0, G)),
                            kc == 0,
                            kc == 5,
                        )
                nc.vector.tensor_scalar_add(QG01, psq, bias["qg"][:, 0:1])
                psq2 = psB.tile([64, G], f32, tag="small", name="psqg2")
                for kc in range(6):
                    mm(psq2, w6["qg"][:, kc, 128:192], xt(kc, slice(0, G)), kc == 0, kc == 5)
                nc.vector.tensor_scalar_add(qg2, psq2, bias["qg"][0:64, 1:2])
                # replicate v-global rows (chunk 0, slots 0/2/4, incl. ones
                # col) to partition offsets {0,32,64} for the sel-PV matmul
                for h in range(HPC):
                    nc.sync.dma_start(
                        out=vg3[32 * h : 32 * h + G, :], in_=vall[0:G, 0, 2 * h, :]
                    )

        # ---- banded local attention block ----
        # Each 128-query half only consumes 5 of the block's 6 kpos chunks,
        # so the two half-specific edge chunks (c=0 -> half 0 / c=5 ->
        # half 1) are computed at N=128 and share score slot 0.
        mask_rr = [0]

        # block 0's staging tile persists: its rows 0..15 are overwritten
        # with the global-row outputs at the very end
        osb3_0 = ph.tile([128, 2, HPC, HD], f32, tag="osb3_0", name="osb3_0")

        def band_block(t):
            # one output staging tile per block: [q mod 128, half, head, d];
            # a single batched DMA writes all 256 rows x 192 cols after the
            # three heads finish
            osb3 = osb3_0 if t == 0 else sbS.tile(
                [128, 2, HPC, HD], f32, tag="osb3", name="osb3"
            )
            cl, ch = _chunk_range(t)
            bexps = {}

            def qk_part(h):
                sc_ps = psA.tile([128, 5, 256], f32, tag="scores", name="sc_ps")
                for c in range(cl, ch):
                    j = 2 * t - 2 + c
                    if c == 0:
                        dst, qs = sc_ps[:, 0, 0:128], slice(256 * t, 256 * t + 128)
                    elif c == 5:
                        dst, qs = (
                            sc_ps[:, 0, 128:256],
                            slice(256 * t + 128, 256 * t + 256),
                        )
                    else:
                        dst, qs = sc_ps[:, c, :], slice(256 * t, 256 * (t + 1))
                    mm(dst, kTh(h, slice(128 * j, 128 * (j + 1))), qTh(h, qs), True, True)
                bexp = bx.tile([128, 5, 256], bf16, tag="bexp", name="bexp")
                nc.scalar.activation(out=bexp, in_=sc_ps, func=AFexp)
                for c in range(cl, ch):
                    for nm, half in mask_apply[(t, c)]:
                        if nm is None:
                            continue
                        if c == 0:
                            sl, cs = 0, slice(0, 128)
                        elif c == 5:
                            sl, cs = 0, slice(128, 256)
                        else:
                            sl, cs = c, slice(128 * half, 128 * (half + 1))
                        eng = nc.vector if mask_rr[0] % 2 else nc.gpsimd
                        mask_rr[0] += 1
                        eng.tensor_mul(
                            bexp[:, sl, cs], bexp[:, sl, cs], masks_sb[:, midx[nm], :]
                        )
                bexps[h] = bexp

            def pv_part(h):
                bexp = bexps[h]
                for half in range(2):
                    q0 = 256 * t + 128 * half
                    chunks = [
                        c
                        for c in range(cl, ch)
                        if (2 * t + half) - 2 <= 2 * t - 2 + c <= (2 * t + half) + 2
                    ]
                    at = psB.tile([128, HD + 1], f32, tag="small", name="at")
                    for ci_, c in enumerate(chunks):
                        j = 2 * t - 2 + c
                        if c == 0:
                            sl, cs = 0, slice(0, 128)
                        elif c == 5:
                            sl, cs = 0, slice(128, 256)
                        else:
                            sl, cs = c, slice(128 * half, 128 * (half + 1))
                        mm(at, bexp[:, sl, cs], vall[:, j, 2 * h, :], ci_ == 0, False)
                    mm(
                        at,
                        selexp3[32 * h : 32 * h + G, q0 : q0 + 128],
                        vg3[32 * h : 32 * h + G, :],
                        False,
                        True,
                    )
                    rec = sbS.tile([128, 1], f32, tag="rec", name="rec")
                    nc.vector.reciprocal(rec, at[:, HD : HD + 1])
                    nc.vector.tensor_scalar_mul(osb3[:, half, h, :], at[:, 0:HD], rec)

            # tensor-queue order QK0 QK1 PV0 QK2 PV1 PV2: each PV's
            # exp+mask latency hides behind the next head's QK, and at most
            # two score tiles are live (psA bufs=2)
            qk_part(0)
            qk_part(1)
            pv_part(0)
            qk_part(2)
            pv_part(1)
            pv_part(2)
            if t == 0:
                # rows 0..15 wait for the global-row outputs; ship the rest
                nc.sync.dma_start(
                    out=out_d[G:128, :], in_=osb3[G:128, 0, :, :]
                )
                nc.sync.dma_start(
                    out=out_d[128:256, :], in_=osb3[:, 1, :, :]
                )
            else:
                nc.sync.dma_start(
                    out=out_d[256 * t : 256 * (t + 1), :].rearrange(
                        "(f p) c -> p f c", p=128
                    ),
                    in_=osb3,
                )

        # ---- schedule: interleave band blocks with projection s-tiles ----
        proj_stile(0)
        proj_stile(1)
        band_block(0)
        band_block(1)
        band_block(2)
        for s in range(2, 8):
            proj_stile(s)
            band_block(2 * s - 1)
            band_block(2 * s)

        # ---- global-token rows: full attention with qg/kg/vg ----
        for h in range(HPC):
            gps = psB.tile([128, NKC, G], f32, tag="small", name="gps")
            for c in range(NKC):
                mm(
                    gps[:, c, :],
                    kgh(h, slice(128 * c, 128 * (c + 1))),
                    qgh(h),
                    True,
                    True,
                )
            nc.scalar.activation(out=eg[h], in_=gps, func=AFexp)
            ops = psB.tile([G, HD + 1], f32, tag="small", name="ops")
            for c in range(NKC):
                mm(ops, eg[h][:, c, :], vall[:, c, 2 * h + 1, :], c == 0, c == NKC - 1)
            recg = sbS.tile([G, 1], f32, tag="recg", name="recg")
            nc.vector.reciprocal(recg, ops[:, HD : HD + 1])
            nc.vector.tensor_scalar_mul(outg[h], ops[:, 0:HD], recg)

        band_block(15)

        # rows 0..15 take the global-row outputs
        for h in range(HPC):
            nc.vector.tensor_copy(out=osb3_0[0:G, 0, h, :], in_=outg[h])
        nc.sync.dma_start(out=out_d[0:G, :], in_=osb3_0[0:G, 0, :, :])

    return nc


def _get_program():
    if "nc" not in _CACHE:
        nc = _build_program()
        nc.finalize()
        _CACHE["nc"] = nc
    return _CACHE["nc"]


def _prep_in_maps(hidden_states, Wq, bq, Wk, bk, Wv, bv, Wqg, bqg, Wkg, bkg, Wvg, bvg):
    hs = np.asarray(hidden_states, dtype=np.float32)
    f32 = np.float32
    bf = ml_dtypes.bfloat16
    f8 = ml_dtypes.float8_e4m3
    in_maps = []
    for c in range(NCORES):
        b = c // 4
        cols = slice(HD * 3 * (c % 4), HD * (3 * (c % 4) + 3))

        def hseg(M, h, scale=1.0):
            return np.asarray(M)[:, cols][:, HD * h : HD * (h + 1)] * scale

        def bseg(v, h, scale=1.0):
            return (np.asarray(v)[cols][HD * h : HD * (h + 1)] * scale).astype(f32)

        def bbast(v):
            # [192] -> broadcast [128, 3, 64]
            a = np.asarray(v)[cols].reshape(HPC, HD).astype(f32)
            return np.ascontiguousarray(np.broadcast_to(a[None], (128, HPC, HD)))

        xTc = np.ascontiguousarray(hs[b].T)
        wqk = np.concatenate(
            [
                hseg(Wq, 0, SCALE), hseg(Wq, 1, SCALE),
                hseg(Wk, 0), hseg(Wk, 1),
                hseg(Wq, 2, SCALE), hseg(Wk, 2),
            ],
            axis=1,
        )
        bqk = np.stack(
            [
                np.concatenate([bseg(bq, 0, SCALE), bseg(bq, 1, SCALE)]),
                np.concatenate([bseg(bk, 0), bseg(bk, 1)]),
                np.concatenate([bseg(bq, 2, SCALE), bseg(bk, 2)]),
            ],
            axis=1,
        )
        bkg2 = np.stack(
            [
                np.concatenate([bseg(bkg, 0), bseg(bkg, 1)]),
                np.concatenate([bseg(bkg, 2), np.zeros(HD, f32)]),
            ],
            axis=1,
        )
        bqg2 = np.stack(
            [
                np.concatenate([bseg(bqg, 0, SCALE), bseg(bqg, 1, SCALE)]),
                np.concatenate([bseg(bqg, 2, SCALE), np.zeros(HD, f32)]),
            ],
            axis=1,
        )
        in_maps.append(
            {
                "xT": xTc.astype(bf),
                "x8T": xTc.astype(f8),
                "Wqk": np.ascontiguousarray(wqk).astype(bf),
                "W8kg": np.ascontiguousarray(np.asarray(Wkg)[:, cols] * FP8S).astype(f8),
                "Wv": np.ascontiguousarray(np.asarray(Wv)[:, cols]).astype(bf),
                "W8vg": np.ascontiguousarray(np.asarray(Wvg)[:, cols] * FP8S).astype(f8),
                "Wqg": np.ascontiguousarray(np.asarray(Wqg)[:, cols] * SCALE).astype(bf),
                "b_qk": np.ascontiguousarray(bqk),
                "b_kg": np.ascontiguousarray(bkg2),
                "b_qg": np.ascontiguousarray(bqg2),
                "b_v": bbast(bv),
                "b_vg": bbast(bvg),
            }
        )
    return in_maps


def kernel(
    hidden_states,
    Wq,
    bq,
    Wk,
    bk,
    Wv,
    bv,
    Wqg,
    bqg,
    Wkg,
    bkg,
    Wvg,
    bvg,
    n_global,
):
    from concourse.bass_utils import run_bass_kernel_spmd

    assert int(n_global) == G
    nc = _get_program()
    in_maps = _prep_in_maps(
        hidden_states, Wq, bq, Wk, bk, Wv, bv, Wqg, bqg, Wkg, bkg, Wvg, bvg
    )
    res = run_bass_kernel_spmd(nc, in_maps, list(range(NCORES)))
    out = np.zeros((B, S, Dm), np.float32)
    for c in range(NCORES):
        b = c // 4
        cols = slice(HD * 3 * (c % 4), HD * (3 * (c % 4) + 3))
        out[b, :, cols] = res.results[c]["out"]
    return out


# revision 48
# speedup vs baseline: 1.0080x; 1.0080x over previous
"""Longformer self-attention Trainium2 kernel (8-core SPMD).

Sharding: core c handles batch b = c//4 and heads [3*(c%4), 3*(c%4)+3).
Each core receives pre-sliced/augmented inputs and computes [4096, 192]
(its 3 heads' output dims); the host reassembles [2, 4096, 768].

Device-side math per core (heads h in 0..3, all layouts chosen so no
on-device transposes are needed):
  - xT [768, 4096] = hidden[b].T; q-scale folded into Wq/Wqg on host.
  - q/k projections packed into one [768, 384] weight (column order
    q0,q1,k0,k1,q2,k2) so PSUM tiles are full 128 rows and evacuate
    with full-lane DVE ops; heads 0/1 of each projection live stacked
    in one [128, S] SBUF tile (head h at partition base 64*(h%2), so
    every per-head matmul has lhsT/rhs at matching partition bases).
  - kg/vg (only consumed by the 16 global-token rows, whose softmax
    averages over all 4096 keys) are computed in fp8e4m3 with the
    DoubleRow perf mode (2 contraction chunks per instruction = 2x
    fewer PE instructions). Weights are pre-scaled by 64 on the host to
    sit in the e4m3 normal range; the 1/64 descale is folded into the
    bias-add evacuation.
  - Band scores computed transposed: sT[kpos, q]. Each 128-query half
    consumes only 5 kpos chunks, so the half-specific edge chunks
    (c=0 -> half 0, c=5 -> half 1) are computed at N=128 and share
    score slot 0 of a [128, 5, 256] PSUM tile.
  - exp() without max subtraction (logits are O(0.3)); band-validity
    and global-exclusion masks are [128, 128] triangles applied
    multiplicatively after the exp, split across DVE and gpsimd.
  - Global columns (sel): the three heads' [16, S] score tiles are
    packed at partition offsets {0, 32, 64} of one [96, S] tensor via
    matmul tile positioning so the exp runs on 96 lanes instead of 16.
  - PV: attn[q, 0:64] and the softmax denominator (ones column of v)
    come out of one accumulated PSUM [128, 65]; normalize = reciprocal
    + mul.
  - Band block t only needs projection s-tiles <= ceil(t/2), so blocks
    2s-1 and 2s are interleaved right after s-tile s: the band's
    scalar/DVE-heavy pipeline fills the projection phase's DMA/evac
    stalls and smooths tensor-engine utilization (the HW power governor
    halves the PE clock when utilization stays pinned near 100%).
  - Global-token rows (0..15) use the qg/kg/vg projections with the
    same transposed-score trick; block 0 (whose rows 0..15 they
    overwrite) runs last.
"""

import sys

sys.path.insert(0, "/opt/trn_rl_repo")

import numpy as np
import ml_dtypes

B, S, Dm, H, WIN, G, HD = 2, 4096, 768, 12, 256, 16, 64
HPC = 3            # heads per core
NCORES = 8
DPC = HPC * HD     # 192 output dims per core
NB = S // WIN      # 16 query blocks
NKC = S // 128     # 32 kpos chunks of 128
SCALE = 1.0 / 8.0  # 1/sqrt(64)
FP8S = 64.0        # fp8 weight pre-scale (host) / descale (evacuation)
QS8 = 512.0        # fp8 pre-scale for q columns (Wq*SCALE has std 0.0025)
KS8 = 64.0         # fp8 pre-scale for k columns

_CACHE = {}


def _mask_classes():
    """Multiplicative {0,1} masks in transposed-score orientation
    [kpos_local p, q_local r (within a 128-query half)], applied to
    exp(scores). Keep (1.0) iff the slot is band-valid and not a global
    key; masked slots contribute exactly 0 to the reference softmax
    (exp(-inf) / exp(x - 10000) both underflow to 0).

    Each 128-query half i (q = 128i + r) consumes kpos chunks
    j = i-2 .. i+2. Only the edge chunks need masks: j = i-2 keeps
    p >= r (lower triangle), j = i+2 keeps p <= r; chunk j = 0
    additionally excludes the global keys (p >= G). Interior chunks are
    fully valid. Returns {name: [128, 128] mask}, plus a per-(t, c)
    application list [(name, half)] verified against the reference
    condition.
    """
    p = np.arange(128)[:, None]
    r = np.arange(128)[None, :]
    classes = {
        "lowT": (p >= r).astype(np.float32),
        "upT": (p <= r).astype(np.float32),
        "lowTg16": ((p >= r) & (p >= G)).astype(np.float32),
        "g16": (p >= G).astype(np.float32) * np.ones((128, 128), np.float32),
    }

    def ref_keep(t, c, half):
        # reference validity of chunk c's slots for query half (t, half)
        kpos = (2 * t - 2 + c) * 128 + p
        i = 256 * t + 128 * half + r
        return (np.abs(kpos - i) <= WIN) & (kpos >= 0) & (kpos < S) & (kpos >= G)

    # application list per (t, c): [(class_name or None, half), ...]
    apply = {}
    for t in range(NB):
        cl, ch = _chunk_range(t)
        for c in range(cl, ch):
            j = 2 * t - 2 + c
            ents = []
            for half in range(2):
                i = 2 * t + half
                if not (i - 2 <= j <= i + 2):
                    continue  # this half never consumes chunk c
                if j == i - 2:
                    nm = "lowTg16" if j == 0 else "lowT"
                elif j == i + 2:
                    nm = "upT"
                elif j == 0:
                    nm = "g16"
                else:
                    nm = None
                if nm is not None:
                    assert np.array_equal(
                        classes[nm].astype(bool), ref_keep(t, c, half)
                    ), (t, c, half, nm)
                else:
                    assert np.all(ref_keep(t, c, half)), (t, c, half)
                ents.append((nm, half))
            apply[(t, c)] = ents
    return classes, apply


def _chunk_range(t):
    if t == 0:
        return 2, 6
    if t == NB - 1:
        return 0, 4
    return 0, 6


def _patch_drain_and_barrier():
    """The walrus build in this container rejects >1 sync-wait on the CTRL
    (Drain) instruction that TileContext emits at exit ("Too many sync wait
    commands"). Split the waits: keep one on the drain, emit the rest as
    explicit single-sem wait_ge instructions on the sync engine before the
    barrier. Semantics preserved: all sems still quiesce before the
    sem-clear + barrier."""
    import concourse.tile as tile
    from concourse import mybir
    from concourse.vector_clock import ScopedClock

    if getattr(tile.TileContext, "_ant_drain_patch", False):
        return

    def _drain_and_barrier(self, tick_clock, wait_clock):
        nc = self.nc
        drain_inst = nc.sync.drain()
        wait_clock.add_sem_waits(
            drain_inst.ins, ScopedClock({None: tick_clock.global_clock})
        )
        si = drain_inst.ins.sync_info
        waits = list(si.on_wait) if si is not None else []
        if len(waits) > 1:
            drain_inst.ins.sync_info = mybir.SyncInfo(
                on_wait=[waits[0]], on_update=list(si.on_update)
            )
            allocated = self.sems.allocated()
            by_name = {}
            for key, sem in allocated.items():
                by_name[str(key)] = sem
                nm = getattr(sem, "name", None)
                if nm is not None:
                    by_name[str(nm)] = sem
            for w in waits[1:]:
                sem = by_name[w.ant_name]
                nc.sync.wait_ge(sem, w.wait_value)
        nc.all_engine_barrier()
        assert self.sems is not None
        popped = nc._tile_sem_poison_stack.pop()
        assert popped is self._sem_poison
        nc.clear_and_free_semaphores(list(self.sems.allocated().values()))
        nc.all_engine_barrier()

    tile.TileContext._drain_and_barrier = _drain_and_barrier
    tile.TileContext._ant_drain_patch = True


def _build_program():
    import concourse.bass as bass
    import concourse.tile as tile
    from concourse import bacc, mybir

    _patch_drain_and_barrier()

    f32 = mybir.dt.float32
    bf16 = mybir.dt.bfloat16
    fp8 = mybir.dt.float8e4
    AF = mybir.ActivationFunctionType
    ALU = mybir.AluOpType
    DR = mybir.MatmulPerfMode.DoubleRow

    # Bacc (not plain Bass): its compile() pipeline runs
    # generate_event_semaphores, which splits multi-sem waits — this
    # walrus build allows at most one sync wait per instruction.
    nc = bacc.Bacc(None)

    xT = nc.dram_tensor("xT", [Dm, S], bf16, kind="ExternalInput")
    x8T = nc.dram_tensor("x8T", [Dm, S], fp8, kind="ExternalInput")
    # column order q0,q1,k0,k1,q2,k2 (64 cols each; q cols pre-scaled)
    Wqk = nc.dram_tensor("Wqk", [Dm, 2 * DPC], bf16, kind="ExternalInput")
    W8kg = nc.dram_tensor("W8kg", [Dm, DPC], fp8, kind="ExternalInput")
    Wv = nc.dram_tensor("Wv", [Dm, DPC], bf16, kind="ExternalInput")
    W8vg = nc.dram_tensor("W8vg", [Dm, DPC], fp8, kind="ExternalInput")
    Wqg = nc.dram_tensor("Wqg", [Dm, DPC], bf16, kind="ExternalInput")
    # stacked bias columns: col layout matches the packed PSUM tiles
    b_qk = nc.dram_tensor("b_qk", [128, 3], f32, kind="ExternalInput")
    b_kg = nc.dram_tensor("b_kg", [128, 2], f32, kind="ExternalInput")
    b_qg = nc.dram_tensor("b_qg", [128, 2], f32, kind="ExternalInput")
    # broadcast v/vg biases: [128 partitions, head, 64]
    b_v = nc.dram_tensor("b_v", [128, HPC, HD], f32, kind="ExternalInput")
    b_vg = nc.dram_tensor("b_vg", [128, HPC, HD], f32, kind="ExternalInput")
    out_d = nc.dram_tensor("out", [S, DPC], f32, kind="ExternalOutput")

    classes, mask_apply = _mask_classes()
    mask_names = list(classes.keys())
    mask_np = np.stack([classes[k] for k in mask_names], axis=1)  # [128, 4, 128]
    masks_d = nc.inline_tensor(mask_np.astype(ml_dtypes.bfloat16), name="masks")
    midx = {k: i for i, k in enumerate(mask_names)}

    from contextlib import ExitStack

    with tile.TileContext(nc) as tc, ExitStack() as ctx:
        const = ctx.enter_context(tc.tile_pool(name="const", bufs=1))
        ph = ctx.enter_context(tc.tile_pool(name="ph", bufs=1))
        xpool = ctx.enter_context(tc.tile_pool(name="xpool", bufs=14))
        x8pool = ctx.enter_context(tc.tile_pool(name="x8pool", bufs=3))
        bx = ctx.enter_context(tc.tile_pool(name="bx", bufs=4))
        sbS = ctx.enter_context(tc.tile_pool(name="sbS", bufs=6))
        psA = ctx.enter_context(tc.tile_pool(name="psA", bufs=2, space="PSUM"))
        psB = ctx.enter_context(tc.tile_pool(name="psB", bufs=2, space="PSUM"))

        # issue the first projection group's operands first (Wqk chunk 0,
        # x chunk 0) so the PE starts within ~1us of kernel entry
        wqk = const.tile([128, 6, 2 * DPC], bf16, tag="wqk", name="wqk")
        nc.sync.dma_start(out=wqk[:, 0, :], in_=Wqk[0:128, :])
        nc.sync.dma_start(out=wqk[:, 1, :], in_=Wqk[128:256, :])

        def x_tiles(ssl, splits):
            # bf16 x chunks as len(splits) tiles; returns accessor kc -> AP
            tiles, offs = [], []
            k0 = 0
            for n in splits:
                t = xpool.tile([128, n, 512], bf16, tag="xt", name="xt")
                nc.sync.dma_start(
                    out=t,
                    in_=xT[128 * k0 : 128 * (k0 + n), ssl].rearrange(
                        "(c p) s -> p c s", p=128
                    ),
                )
                tiles.append(t)
                offs.append(k0)
                k0 += n
            def xtc(kc, cs=slice(0, 512)):
                for t, o, n in zip(tiles, offs, splits):
                    if o <= kc < o + n:
                        return t[:, kc - o, cs]
                raise KeyError(kc)
            return xtc

        def x8_tile(ssl):
            t8 = x8pool.tile([128, 6, 512], fp8, tag="xt8", name="xt8")
            nc.sync.dma_start(
                out=t8, in_=x8T[:, ssl].rearrange("(c p) s -> p c s", p=128)
            )
            return t8

        xt0 = x_tiles(slice(0, 512), (1, 1, 2, 2))
        nc.sync.dma_start(
            out=wqk[:, 2:6, :],
            in_=Wqk[256:768, :].rearrange("(c p) d -> p c d", p=128),
        )
        xt80 = x8_tile(slice(0, 512))

        # ---- remaining constants to SBUF ----
        w6 = {}
        for nm, dram, width, dt in (
            ("kg", W8kg, DPC, fp8),
            ("v", Wv, DPC, bf16),
            ("vg", W8vg, DPC, fp8),
            ("qg", Wqg, DPC, bf16),
        ):
            w6[nm] = const.tile([128, 6, width], dt, tag=f"w6{nm}", name=f"w6{nm}")
            nc.sync.dma_start(
                out=w6[nm], in_=dram[:, :].rearrange("(c p) d -> p c d", p=128)
            )
        bias = {}
        for nm, dram, w in (("qk", b_qk, 3), ("kg", b_kg, 2), ("qg", b_qg, 2)):
            bias[nm] = const.tile([128, w], f32, tag=f"b{nm}", name=f"b{nm}")
            nc.sync.dma_start(out=bias[nm], in_=dram[:])
        bv_sb = const.tile([128, HPC, HD], f32, tag="bv", name="bv_sb")
        nc.sync.dma_start(out=bv_sb, in_=b_v[:])
        bvg_sb = const.tile([128, HPC, HD], f32, tag="bvg", name="bvg_sb")
        nc.sync.dma_start(out=bvg_sb, in_=b_vg[:])
        masks_sb = const.tile([128, 4, 128], bf16, tag="masks", name="masks_sb")
        nc.sync.dma_start(out=masks_sb, in_=masks_d[:])

        # ---- persistent per-head tensors (heads 0/1 stacked per tile) ----
        P0 = ph.tile([128, S], bf16, tag="P0", name="P0")   # [q0; q1]
        P1 = ph.tile([128, S], bf16, tag="P1", name="P1")   # [k0; k1]
        q2 = ph.tile([64, S], bf16, tag="q2", name="q2")
        k2 = ph.tile([64, S], bf16, tag="k2", name="k2")
        KG01 = ph.tile([128, S], bf16, tag="KG01", name="KG01")
        kg2 = ph.tile([64, S], bf16, tag="kg2", name="kg2")
        QG01 = ph.tile([128, G], bf16, tag="QG01", name="QG01")
        qg2 = ph.tile([64, G], bf16, tag="qg2", name="qg2")

        def qTh(h, cs):
            return P0[64 * h : 64 * h + 64, cs] if h < 2 else q2[:, cs]

        def kTh(h, cs):
            return P1[64 * h : 64 * h + 64, cs] if h < 2 else k2[:, cs]

        def kgh(h, cs):
            return KG01[64 * h : 64 * h + 64, cs] if h < 2 else kg2[:, cs]

        def qgh(h):
            return QG01[64 * h : 64 * h + 64, :] if h < 2 else qg2[:, :]

        # v/vg interleaved with ones column: [:, chunk, 2h+0, :] = v head h,
        # [:, chunk, 2h+1, :] = vg head h ([:, :, :, 64] = 1.0)
        vall = ph.tile([128, NKC, 2 * HPC, HD + 1], bf16, tag="vall", name="vall")
        nc.vector.memset(vall[:, :, :, HD : HD + 1], 1.0)
        # three heads' global-column exp'd scores packed at partition
        # offsets {0, 32, 64}: rows 32h..32h+16 = head h's [16, S]
        selexp3 = ph.tile([96, S], bf16, tag="selexp3", name="selexp3")
        # v-global rows replicated at the same offsets for the PV matmul
        vg3 = ph.tile([96, HD + 1], bf16, tag="vg3", name="vg3")
        eg = [
            ph.tile([128, NKC, G], bf16, tag=f"eg{h}", name=f"eg{h}")
            for h in range(HPC)
        ]
        outg = [ph.tile([G, HD], f32, tag=f"outg{h}", name=f"outg{h}") for h in range(HPC)]

        def mm(out, lhsT, rhs, start, stop):
            nc.tensor.matmul(out, lhsT, rhs, start=start, stop=stop)

        AFexp = AF.Exp

        def vall_slot_ap(ci, par, width=HD):
            # [128, h, d] AP over vall slots (par=0: v slots 0/2/4;
            # par=1: vg slots 1/3/5) of kpos chunk ci
            return bass.AP(
                tensor=vall.tensor,
                offset=vall.offset + (ci * 2 * HPC + par) * (HD + 1),
                ap=[vall.ap[0], [2 * (HD + 1), HPC], [1, width]],
            )

        # ---- projection s-tile body ----
        def proj_stile(st):
            ssl = slice(512 * st, 512 * (st + 1))
            if st == 0:
                xt, xt8 = xt0, xt80
            else:
                xt8 = x8_tile(ssl)
                xt = x_tiles(ssl, (3, 3))

            # q/k packed: transposed layout, W stationary, 3 full PSUM
            # tiles [q0;q1], [k0;k1], [q2;k2]
            for dc in range(3):
                d0 = 128 * dc
                ps = psB.tile([128, 512], f32, tag="small", name="psqk")
                for kc in range(6):
                    mm(ps, wqk[:, kc, d0 : d0 + 128], xt(kc), kc == 0, kc == 5)
                if dc == 0:
                    nc.vector.tensor_scalar_add(P0[:, ssl], ps, bias["qk"][:, 0:1])
                elif dc == 1:
                    nc.vector.tensor_scalar_add(P1[:, ssl], ps, bias["qk"][:, 1:2])
                else:
                    nc.vector.tensor_scalar_add(
                        q2[:, ssl], ps[0:64, :], bias["qk"][0:64, 2:3]
                    )
                    nc.vector.tensor_scalar_add(
                        k2[:, ssl], ps[64:128, :], bias["qk"][64:128, 2:3]
                    )

            # kg: fp8 DoubleRow, transposed layout, W stationary
            for ti, (d0, d1) in enumerate(((0, 128), (128, 192))):
                ps = psB.tile([d1 - d0, 512], f32, tag="small", name="pskg")
                for p in range(3):
                    nc.tensor.matmul(
                        ps,
                        w6["kg"][:, 2 * p : 2 * p + 2, d0:d1],
                        xt8[:, 2 * p : 2 * p + 2, :],
                        start=(p == 0),
                        stop=(p == 2),
                        perf_mode=DR,
                    )
                dst = KG01[:, ssl] if ti == 0 else kg2[:, ssl]
                nc.vector.tensor_scalar(
                    dst,
                    ps,
                    1.0 / FP8S,
                    bias["kg"][0 : d1 - d0, ti : ti + 1],
                    ALU.mult,
                    ALU.add,
                )

            # v: natural layout, xT chunks stationary (bf16)
            for sc in range(4):
                ci = 4 * st + sc
                msl = slice(128 * sc, 128 * (sc + 1))
                psv = psB.tile([128, DPC], f32, tag="small", name="psv")
                for kc in range(6):
                    mm(psv, xt(kc, msl), w6["v"][:, kc, :], kc == 0, kc == 5)
                nc.vector.tensor_add(
                    vall_slot_ap(ci, 0),
                    psv[:, :].rearrange("p (h d) -> p h d", h=HPC),
                    bv_sb,
                )

                # vg: natural layout, fp8 DoubleRow, xT chunks stationary
                psg = psB.tile([128, DPC], f32, tag="small", name="psvg")
                for p in range(3):
                    nc.tensor.matmul(
                        psg,
                        xt8[:, 2 * p : 2 * p + 2, msl],
                        w6["vg"][:, 2 * p : 2 * p + 2, :],
                        start=(p == 0),
                        stop=(p == 2),
                        perf_mode=DR,
                    )
                nc.vector.scalar_tensor_tensor(
                    vall_slot_ap(ci, 1),
                    psg[:, :].rearrange("p (h d) -> p h d", h=HPC),
                    1.0 / FP8S,
                    bvg_sb,
                    ALU.mult,
                    ALU.add,
                )

            # global columns for this s-tile: sel = q . k[:G], all heads
            # packed into one [96, 512] PSUM tile so the exp uses 96 lanes
            sps = psB.tile([96, 512], f32, tag="small", name="sps")
            for h in range(HPC):
                mm(
                    sps[32 * h : 32 * h + G, :],
                    kTh(h, slice(0, G)),
                    qTh(h, ssl),
                    True,
                    True,
                )
            nc.scalar.activation(out=selexp3[:, ssl], in_=sps, func=AFexp)

            if st == 0:
                # qg: heads 0/1 into one [128, G] PSUM tile, head 2 separate
                psq = psB.tile([128, G], f32, tag="small", name="psqg")
                for mq in range(2):
                    for kc in range(6):
                        mm(
                            psq[64 * mq : 64 * mq + 64, :],
                            w6["qg"][:, kc, 64 * mq : 64 * mq + 64],
                            xt(kc, slice(0, G)),
                            kc == 0,
                            kc == 5,
                        )
                nc.vector.tensor_scalar_add(QG01, psq, bias["qg"][:, 0:1])
                psq2 = psB.tile([64, G], f32, tag="small", name="psqg2")
                for kc in range(6):
                    mm(psq2, w6["qg"][:, kc, 128:192], xt(kc, slice(0, G)), kc == 0, kc == 5)
                nc.vector.tensor_scalar_add(qg2, psq2, bias["qg"][0:64, 1:2])
                # replicate v-global rows (chunk 0, slots 0/2/4, incl. ones
                # col) to partition offsets {0,32,64} for the sel-PV matmul
                for h in range(HPC):
                    nc.sync.dma_start(
                        out=vg3[32 * h : 32 * h + G, :], in_=vall[0:G, 0, 2 * h, :]
                    )

        # ---- banded local attention block ----
        # Each 128-query half only consumes 5 of the block's 6 kpos chunks,
        # so the two half-specific edge chunks (c=0 -> half 0 / c=5 ->
        # half 1) are computed at N=128 and share score slot 0.
        mask_rr = [0]

        # block 0's staging tile persists: its rows 0..15 are overwritten
        # with the global-row outputs at the very end
        osb3_0 = ph.tile([128, 2, HPC, HD], f32, tag="osb3_0", name="osb3_0")

        def band_block(t):
            # one output staging tile per block: [q mod 128, half, head, d];
            # a single batched DMA writes all 256 rows x 192 cols after the
            # three heads finish
            osb3 = osb3_0 if t == 0 else sbS.tile(
                [128, 2, HPC, HD], f32, tag="osb3", name="osb3"
            )
            cl, ch = _chunk_range(t)
            bexps = {}

            def qk_part(h):
                sc_ps = psA.tile([128, 5, 256], f32, tag="scores", name="sc_ps")
                for c in range(cl, ch):
                    j = 2 * t - 2 + c
                    if c == 0:
                        dst, qs = sc_ps[:, 0, 0:128], slice(256 * t, 256 * t + 128)
                    elif c == 5:
                        dst, qs = (
                            sc_ps[:, 0, 128:256],
                            slice(256 * t + 128, 256 * t + 256),
                        )
                    else:
                        dst, qs = sc_ps[:, c, :], slice(256 * t, 256 * (t + 1))
                    mm(dst, kTh(h, slice(128 * j, 128 * (j + 1))), qTh(h, qs), True, True)
                bexp = bx.tile([128, 5, 256], bf16, tag="bexp", name="bexp")
                nc.scalar.activation(out=bexp, in_=sc_ps, func=AFexp)
                for c in range(cl, ch):
                    for nm, half in mask_apply[(t, c)]:
                        if nm is None:
                            continue
                        if c == 0:
                            sl, cs = 0, slice(0, 128)
                        elif c == 5:
                            sl, cs = 0, slice(128, 256)
                        else:
                            sl, cs = c, slice(128 * half, 128 * (half + 1))
                        eng = nc.vector if mask_rr[0] % 2 else nc.gpsimd
                        mask_rr[0] += 1
                        eng.tensor_mul(
                            bexp[:, sl, cs], bexp[:, sl, cs], masks_sb[:, midx[nm], :]
                        )
                bexps[h] = bexp

            def pv_part(h):
                bexp = bexps[h]
                for half in range(2):
                    q0 = 256 * t + 128 * half
                    chunks = [
                        c
                        for c in range(cl, ch)
                        if (2 * t + half) - 2 <= 2 * t - 2 + c <= (2 * t + half) + 2
                    ]
                    at = psB.tile([128, HD + 1], f32, tag="small", name="at")
                    for ci_, c in enumerate(chunks):
                        j = 2 * t - 2 + c
                        if c == 0:
                            sl, cs = 0, slice(0, 128)
                        elif c == 5:
                            sl, cs = 0, slice(128, 256)
                        else:
                            sl, cs = c, slice(128 * half, 128 * (half + 1))
                        mm(at, bexp[:, sl, cs], vall[:, j, 2 * h, :], ci_ == 0, False)
                    mm(
                        at,
                        selexp3[32 * h : 32 * h + G, q0 : q0 + 128],
                        vg3[32 * h : 32 * h + G, :],
                        False,
                        True,
                    )
                    rec = sbS.tile([128, 1], f32, tag="rec", name="rec")
                    nc.vector.reciprocal(rec, at[:, HD : HD + 1])
                    nc.vector.tensor_scalar_mul(osb3[:, half, h, :], at[:, 0:HD], rec)

            # per-head QK -> exp/mask -> PV; latency hides across the
            # adjacent interleaved blocks and projection s-tiles
            for h in range(HPC):
                qk_part(h)
                pv_part(h)
            if t == 0:
                # rows 0..15 wait for the global-row outputs; ship the rest
                nc.sync.dma_start(
                    out=out_d[G:128, :], in_=osb3[G:128, 0, :, :]
                )
                nc.sync.dma_start(
                    out=out_d[128:256, :], in_=osb3[:, 1, :, :]
                )
            else:
                nc.sync.dma_start(
                    out=out_d[256 * t : 256 * (t + 1), :].rearrange(
                        "(f p) c -> p f c", p=128
                    ),
                    in_=osb3,
                )

        # ---- schedule: interleave band blocks with projection s-tiles ----
        proj_stile(0)
        proj_stile(1)
        band_block(0)
        band_block(1)
        band_block(2)
        for s in range(2, 8):
            proj_stile(s)
            band_block(2 * s - 1)
            band_block(2 * s)

        # ---- global-token rows: full attention with qg/kg/vg ----
        for h in range(HPC):
            gps = psB.tile([128, NKC, G], f32, tag="small", name="gps")
            for c in range(NKC):
                mm(
                    gps[:, c, :],
                    kgh(h, slice(128 * c, 128 * (c + 1))),
                    qgh(h),
                    True,
                    True,
                )
            nc.scalar.activation(out=eg[h], in_=gps, func=AFexp)
            ops = psB.tile([G, HD + 1], f32, tag="small", name="ops")
            for c in range(NKC):
                mm(ops, eg[h][:, c, :], vall[:, c, 2 * h + 1, :], c == 0, c == NKC - 1)
            recg = sbS.tile([G, 1], f32, tag="recg", name="recg")
            nc.vector.reciprocal(recg, ops[:, HD : HD + 1])
            nc.vector.tensor_scalar_mul(outg[h], ops[:, 0:HD], recg)

        band_block(15)

        # rows 0..15 take the global-row outputs
        for h in range(HPC):
            nc.vector.tensor_copy(out=osb3_0[0:G, 0, h, :], in_=outg[h])
        nc.sync.dma_start(out=out_d[0:G, :], in_=osb3_0[0:G, 0, :, :])

    return nc


def _get_program():
    if "nc" not in _CACHE:
        nc = _build_program()
        nc.finalize()
        _CACHE["nc"] = nc
    return _CACHE["nc"]


def _prep_in_maps(hidden_states, Wq, bq, Wk, bk, Wv, bv, Wqg, bqg, Wkg, bkg, Wvg, bvg):
    hs = np.asarray(hidden_states, dtype=np.float32)
    f32 = np.float32
    bf = ml_dtypes.bfloat16
    f8 = ml_dtypes.float8_e4m3
    in_maps = []
    for c in range(NCORES):
        b = c // 4
        cols = slice(HD * 3 * (c % 4), HD * (3 * (c % 4) + 3))

        def hseg(M, h, scale=1.0):
            return np.asarray(M)[:, cols][:, HD * h : HD * (h + 1)] * scale

        def bseg(v, h, scale=1.0):
            return (np.asarray(v)[cols][HD * h : HD * (h + 1)] * scale).astype(f32)

        def bbast(v):
            # [192] -> broadcast [128, 3, 64]
            a = np.asarray(v)[cols].reshape(HPC, HD).astype(f32)
            return np.ascontiguousarray(np.broadcast_to(a[None], (128, HPC, HD)))

        xTc = np.ascontiguousarray(hs[b].T)
        wqk = np.concatenate(
            [
                hseg(Wq, 0, SCALE), hseg(Wq, 1, SCALE),
                hseg(Wk, 0), hseg(Wk, 1),
                hseg(Wq, 2, SCALE), hseg(Wk, 2),
            ],
            axis=1,
        )
        bqk = np.stack(
            [
                np.concatenate([bseg(bq, 0, SCALE), bseg(bq, 1, SCALE)]),
                np.concatenate([bseg(bk, 0), bseg(bk, 1)]),
                np.concatenate([bseg(bq, 2, SCALE), bseg(bk, 2)]),
            ],
            axis=1,
        )
        bkg2 = np.stack(
            [
                np.concatenate([bseg(bkg, 0), bseg(bkg, 1)]),
                np.concatenate([bseg(bkg, 2), np.zeros(HD, f32)]),
            ],
            axis=1,
        )
        bqg2 = np.stack(
            [
                np.concatenate([bseg(bqg, 0, SCALE), bseg(bqg, 1, SCALE)]),
                np.concatenate([bseg(bqg, 2, SCALE), np.zeros(HD, f32)]),
            ],
            axis=1,
        )
        in_maps.append(
            {
                "xT": xTc.astype(bf),
                "x8T": xTc.astype(f8),
                "Wqk": np.ascontiguousarray(wqk).astype(bf),
                "W8kg": np.ascontiguousarray(np.asarray(Wkg)[:, cols] * FP8S).astype(f8),
                "Wv": np.ascontiguousarray(np.asarray(Wv)[:, cols]).astype(bf),
                "W8vg": np.ascontiguousarray(np.asarray(Wvg)[:, cols] * FP8S).astype(f8),
                "Wqg": np.ascontiguousarray(np.asarray(Wqg)[:, cols] * SCALE).astype(bf),
                "b_qk": np.ascontiguousarray(bqk),
                "b_kg": np.ascontiguousarray(bkg2),
                "b_qg": np.ascontiguousarray(bqg2),
                "b_v": bbast(bv),
                "b_vg": bbast(bvg),
            }
        )
    return in_maps


def kernel(
    hidden_states,
    Wq,
    bq,
    Wk,
    bk,
    Wv,
    bv,
    Wqg,
    bqg,
    Wkg,
    bkg,
    Wvg,
    bvg,
    n_global,
):
    from concourse.bass_utils import run_bass_kernel_spmd

    assert int(n_global) == G
    nc = _get_program()
    in_maps = _prep_in_maps(
        hidden_states, Wq, bq, Wk, bk, Wv, bv, Wqg, bqg, Wkg, bkg, Wvg, bvg
    )
    res = run_bass_kernel_spmd(nc, in_maps, list(range(NCORES)))
    out = np.zeros((B, S, Dm), np.float32)
    for c in range(NCORES):
        b = c // 4
        cols = slice(HD * 3 * (c % 4), HD * (3 * (c % 4) + 3))
        out[b, :, cols] = res.results[c]["out"]
    return out


# revision 54
# speedup vs baseline: 1.0240x; 1.0159x over previous
"""Longformer self-attention Trainium2 kernel (8-core SPMD).

Sharding: core c handles batch b = c//4 and heads [3*(c%4), 3*(c%4)+3).
Each core receives pre-sliced/augmented inputs and computes [4096, 192]
(its 3 heads' output dims); the host reassembles [2, 4096, 768].

Device-side math per core (heads h in 0..3, all layouts chosen so no
on-device transposes are needed):
  - xT [768, 4096] = hidden[b].T; q-scale folded into Wq/Wqg on host.
  - q/k projections packed into one [768, 384] weight (column order
    q0,q1,k0,k1,q2,k2) so PSUM tiles are full 128 rows and evacuate
    with full-lane DVE ops; heads 0/1 of each projection live stacked
    in one [128, S] SBUF tile (head h at partition base 64*(h%2), so
    every per-head matmul has lhsT/rhs at matching partition bases).
  - kg/vg (only consumed by the 16 global-token rows, whose softmax
    averages over all 4096 keys) are computed in fp8e4m3 with the
    DoubleRow perf mode (2 contraction chunks per instruction = 2x
    fewer PE instructions). Weights are pre-scaled by 64 on the host to
    sit in the e4m3 normal range; the 1/64 descale is folded into the
    bias-add evacuation.
  - Band scores computed transposed: sT[kpos, q]. Each 128-query half
    consumes only 5 kpos chunks, so the half-specific edge chunks
    (c=0 -> half 0, c=5 -> half 1) are computed at N=128 and share
    score slot 0 of a [128, 5, 256] PSUM tile.
  - exp() without max subtraction (logits are O(0.3)); band-validity
    and global-exclusion masks are [128, 128] triangles applied
    multiplicatively after the exp, split across DVE and gpsimd.
  - Global columns (sel): the three heads' [16, S] score tiles are
    packed at partition offsets {0, 32, 64} of one [96, S] tensor via
    matmul tile positioning so the exp runs on 96 lanes instead of 16.
  - PV: attn[q, 0:64] and the softmax denominator (ones column of v)
    come out of one accumulated PSUM [128, 65]; normalize = reciprocal
    + mul.
  - Band block t only needs projection s-tiles <= ceil(t/2), so blocks
    2s-1 and 2s are interleaved right after s-tile s: the band's
    scalar/DVE-heavy pipeline fills the projection phase's DMA/evac
    stalls and smooths tensor-engine utilization (the HW power governor
    halves the PE clock when utilization stays pinned near 100%).
  - Global-token rows (0..15) use the qg/kg/vg projections with the
    same transposed-score trick; block 0 (whose rows 0..15 they
    overwrite) runs last.
"""

import sys

sys.path.insert(0, "/opt/trn_rl_repo")

import numpy as np
import ml_dtypes

B, S, Dm, H, WIN, G, HD = 2, 4096, 768, 12, 256, 16, 64
HPC = 3            # heads per core
NCORES = 8
DPC = HPC * HD     # 192 output dims per core
NB = S // WIN      # 16 query blocks
NKC = S // 128     # 32 kpos chunks of 128
SCALE = 1.0 / 8.0  # 1/sqrt(64)
FP8S = 64.0        # fp8 weight pre-scale (host) / descale (evacuation)
QS8 = 512.0        # fp8 pre-scale for q columns (Wq*SCALE has std 0.0025)
KS8 = 64.0         # fp8 pre-scale for k columns

_CACHE = {}


def _mask_classes():
    """Multiplicative {0,1} masks in transposed-score orientation
    [kpos_local p, q_local r (within a 128-query half)], applied to
    exp(scores). Keep (1.0) iff the slot is band-valid and not a global
    key; masked slots contribute exactly 0 to the reference softmax
    (exp(-inf) / exp(x - 10000) both underflow to 0).

    Each 128-query half i (q = 128i + r) consumes kpos chunks
    j = i-2 .. i+2. Only the edge chunks need masks: j = i-2 keeps
    p >= r (lower triangle), j = i+2 keeps p <= r; chunk j = 0
    additionally excludes the global keys (p >= G). Interior chunks are
    fully valid. Returns {name: [128, 128] mask}, plus a per-(t, c)
    application list [(name, half)] verified against the reference
    condition.
    """
    p = np.arange(128)[:, None]
    r = np.arange(128)[None, :]
    classes = {
        "lowT": (p >= r).astype(np.float32),
        "upT": (p <= r).astype(np.float32),
        "lowTg16": ((p >= r) & (p >= G)).astype(np.float32),
        "g16": (p >= G).astype(np.float32) * np.ones((128, 128), np.float32),
    }

    def ref_keep(t, c, half):
        # reference validity of chunk c's slots for query half (t, half)
        kpos = (2 * t - 2 + c) * 128 + p
        i = 256 * t + 128 * half + r
        return (np.abs(kpos - i) <= WIN) & (kpos >= 0) & (kpos < S) & (kpos >= G)

    # application list per (t, c): [(class_name or None, half), ...]
    apply = {}
    for t in range(NB):
        cl, ch = _chunk_range(t)
        for c in range(cl, ch):
            j = 2 * t - 2 + c
            ents = []
            for half in range(2):
                i = 2 * t + half
                if not (i - 2 <= j <= i + 2):
                    continue  # this half never consumes chunk c
                if j == i - 2:
                    nm = "lowTg16" if j == 0 else "lowT"
                elif j == i + 2:
                    nm = "upT"
                elif j == 0:
                    nm = "g16"
                else:
                    nm = None
                if nm is not None:
                    assert np.array_equal(
                        classes[nm].astype(bool), ref_keep(t, c, half)
                    ), (t, c, half, nm)
                else:
                    assert np.all(ref_keep(t, c, half)), (t, c, half)
                ents.append((nm, half))
            apply[(t, c)] = ents
    return classes, apply


def _chunk_range(t):
    if t == 0:
        return 2, 6
    if t == NB - 1:
        return 0, 4
    return 0, 6


def _patch_drain_and_barrier():
    """The walrus build in this container rejects >1 sync-wait on the CTRL
    (Drain) instruction that TileContext emits at exit ("Too many sync wait
    commands"). Split the waits: keep one on the drain, emit the rest as
    explicit single-sem wait_ge instructions on the sync engine before the
    barrier. Semantics preserved: all sems still quiesce before the
    sem-clear + barrier."""
    import concourse.tile as tile
    from concourse import mybir
    from concourse.vector_clock import ScopedClock

    if getattr(tile.TileContext, "_ant_drain_patch", False):
        return

    def _drain_and_barrier(self, tick_clock, wait_clock):
        nc = self.nc
        drain_inst = nc.sync.drain()
        wait_clock.add_sem_waits(
            drain_inst.ins, ScopedClock({None: tick_clock.global_clock})
        )
        si = drain_inst.ins.sync_info
        waits = list(si.on_wait) if si is not None else []
        if len(waits) > 1:
            drain_inst.ins.sync_info = mybir.SyncInfo(
                on_wait=[waits[0]], on_update=list(si.on_update)
            )
            allocated = self.sems.allocated()
            by_name = {}
            for key, sem in allocated.items():
                by_name[str(key)] = sem
                nm = getattr(sem, "name", None)
                if nm is not None:
                    by_name[str(nm)] = sem
            for w in waits[1:]:
                sem = by_name[w.ant_name]
                nc.sync.wait_ge(sem, w.wait_value)
        nc.all_engine_barrier()
        assert self.sems is not None
        popped = nc._tile_sem_poison_stack.pop()
        assert popped is self._sem_poison
        nc.clear_and_free_semaphores(list(self.sems.allocated().values()))
        nc.all_engine_barrier()

    tile.TileContext._drain_and_barrier = _drain_and_barrier
    tile.TileContext._ant_drain_patch = True


def _build_program():
    import concourse.bass as bass
    import concourse.tile as tile
    from concourse import bacc, mybir

    _patch_drain_and_barrier()

    f32 = mybir.dt.float32
    bf16 = mybir.dt.bfloat16
    fp8 = mybir.dt.float8e4
    AF = mybir.ActivationFunctionType
    ALU = mybir.AluOpType
    DR = mybir.MatmulPerfMode.DoubleRow

    # Bacc (not plain Bass): its compile() pipeline runs
    # generate_event_semaphores, which splits multi-sem waits — this
    # walrus build allows at most one sync wait per instruction.
    nc = bacc.Bacc(None)

    xT = nc.dram_tensor("xT", [Dm, S], bf16, kind="ExternalInput")
    x8T = nc.dram_tensor("x8T", [Dm, S], fp8, kind="ExternalInput")
    # column order q0,q1,k0,k1,q2,k2 (64 cols each; q cols pre-scaled)
    Wqk = nc.dram_tensor("Wqk", [Dm, 2 * DPC], bf16, kind="ExternalInput")
    W8kg = nc.dram_tensor("W8kg", [Dm, DPC], fp8, kind="ExternalInput")
    Wv = nc.dram_tensor("Wv", [Dm, DPC], bf16, kind="ExternalInput")
    W8vg = nc.dram_tensor("W8vg", [Dm, DPC], fp8, kind="ExternalInput")
    Wqg = nc.dram_tensor("Wqg", [Dm, DPC], bf16, kind="ExternalInput")
    # stacked bias columns: col layout matches the packed PSUM tiles
    b_qk = nc.dram_tensor("b_qk", [128, 3], f32, kind="ExternalInput")
    b_kg = nc.dram_tensor("b_kg", [128, 2], f32, kind="ExternalInput")
    b_qg = nc.dram_tensor("b_qg", [128, 2], f32, kind="ExternalInput")
    # broadcast v/vg biases: [128 partitions, head, 64]
    b_v = nc.dram_tensor("b_v", [128, HPC, HD], f32, kind="ExternalInput")
    b_vg = nc.dram_tensor("b_vg", [128, HPC, HD], f32, kind="ExternalInput")
    out_d = nc.dram_tensor("out", [S, DPC], f32, kind="ExternalOutput")

    classes, mask_apply = _mask_classes()
    mask_names = list(classes.keys())
    mask_np = np.stack([classes[k] for k in mask_names], axis=1)  # [128, 4, 128]
    masks_d = nc.inline_tensor(mask_np.astype(ml_dtypes.bfloat16), name="masks")
    midx = {k: i for i, k in enumerate(mask_names)}

    from contextlib import ExitStack

    with tile.TileContext(nc) as tc, ExitStack() as ctx:
        const = ctx.enter_context(tc.tile_pool(name="const", bufs=1))
        ph = ctx.enter_context(tc.tile_pool(name="ph", bufs=1))
        xpool = ctx.enter_context(tc.tile_pool(name="xpool", bufs=14))
        x8pool = ctx.enter_context(tc.tile_pool(name="x8pool", bufs=3))
        bx = ctx.enter_context(tc.tile_pool(name="bx", bufs=4))
        sbS = ctx.enter_context(tc.tile_pool(name="sbS", bufs=6))
        psA = ctx.enter_context(tc.tile_pool(name="psA", bufs=2, space="PSUM"))
        psB = ctx.enter_context(tc.tile_pool(name="psB", bufs=2, space="PSUM"))

        # issue the first projection group's operands first (Wqk chunk 0,
        # x chunk 0) so the PE starts within ~1us of kernel entry
        wqk = const.tile([128, 6, 2 * DPC], bf16, tag="wqk", name="wqk")
        nc.sync.dma_start(out=wqk[:, 0, :], in_=Wqk[0:128, :])
        nc.sync.dma_start(out=wqk[:, 1, :], in_=Wqk[128:256, :])

        def x_tiles(ssl, splits):
            # bf16 x chunks as len(splits) tiles; returns accessor kc -> AP
            tiles, offs = [], []
            k0 = 0
            for n in splits:
                t = xpool.tile([128, n, 512], bf16, tag="xt", name="xt")
                nc.sync.dma_start(
                    out=t,
                    in_=xT[128 * k0 : 128 * (k0 + n), ssl].rearrange(
                        "(c p) s -> p c s", p=128
                    ),
                )
                tiles.append(t)
                offs.append(k0)
                k0 += n
            def xtc(kc, cs=slice(0, 512)):
                for t, o, n in zip(tiles, offs, splits):
                    if o <= kc < o + n:
                        return t[:, kc - o, cs]
                raise KeyError(kc)
            return xtc

        def x8_tile(ssl):
            t8 = x8pool.tile([128, 6, 512], fp8, tag="xt8", name="xt8")
            nc.sync.dma_start(
                out=t8, in_=x8T[:, ssl].rearrange("(c p) s -> p c s", p=128)
            )
            return t8

        xt0 = x_tiles(slice(0, 512), (1, 1, 2, 2))
        nc.sync.dma_start(
            out=wqk[:, 2:6, :],
            in_=Wqk[256:768, :].rearrange("(c p) d -> p c d", p=128),
        )
        xt80 = x8_tile(slice(0, 512))

        # ---- remaining constants to SBUF ----
        w6 = {}
        for nm, dram, width, dt in (
            ("kg", W8kg, DPC, fp8),
            ("v", Wv, DPC, bf16),
            ("vg", W8vg, DPC, fp8),
            ("qg", Wqg, DPC, bf16),
        ):
            w6[nm] = const.tile([128, 6, width], dt, tag=f"w6{nm}", name=f"w6{nm}")
            nc.sync.dma_start(
                out=w6[nm], in_=dram[:, :].rearrange("(c p) d -> p c d", p=128)
            )
        bias = {}
        for nm, dram, w in (("qk", b_qk, 3), ("kg", b_kg, 2), ("qg", b_qg, 2)):
            bias[nm] = const.tile([128, w], f32, tag=f"b{nm}", name=f"b{nm}")
            nc.sync.dma_start(out=bias[nm], in_=dram[:])
        bv_sb = const.tile([128, HPC, HD], f32, tag="bv", name="bv_sb")
        nc.sync.dma_start(out=bv_sb, in_=b_v[:])
        bvg_sb = const.tile([128, HPC, HD], f32, tag="bvg", name="bvg_sb")
        nc.sync.dma_start(out=bvg_sb, in_=b_vg[:])
        masks_sb = const.tile([128, 4, 128], bf16, tag="masks", name="masks_sb")
        nc.sync.dma_start(out=masks_sb, in_=masks_d[:])

        # ---- persistent per-head tensors (heads 0/1 stacked per tile) ----
        P0 = ph.tile([128, S], bf16, tag="P0", name="P0")   # [q0; q1]
        P1 = ph.tile([128, S], bf16, tag="P1", name="P1")   # [k0; k1]
        q2 = ph.tile([64, S], bf16, tag="q2", name="q2")
        k2 = ph.tile([64, S], bf16, tag="k2", name="k2")
        KG01 = ph.tile([128, S], bf16, tag="KG01", name="KG01")
        kg2 = ph.tile([64, S], bf16, tag="kg2", name="kg2")
        QG01 = ph.tile([128, G], bf16, tag="QG01", name="QG01")
        qg2 = ph.tile([64, G], bf16, tag="qg2", name="qg2")

        def qTh(h, cs):
            return P0[64 * h : 64 * h + 64, cs] if h < 2 else q2[:, cs]

        def kTh(h, cs):
            return P1[64 * h : 64 * h + 64, cs] if h < 2 else k2[:, cs]

        def kgh(h, cs):
            return KG01[64 * h : 64 * h + 64, cs] if h < 2 else kg2[:, cs]

        def qgh(h):
            return QG01[64 * h : 64 * h + 64, :] if h < 2 else qg2[:, :]

        # v/vg interleaved with ones column: [:, chunk, 2h+0, :] = v head h,
        # [:, chunk, 2h+1, :] = vg head h ([:, :, :, 64] = 1.0)
        vall = ph.tile([128, NKC, 2 * HPC, HD + 1], bf16, tag="vall", name="vall")
        nc.vector.memset(vall[:, :, :, HD : HD + 1], 1.0)
        # three heads' global-column exp'd scores packed at partition
        # offsets {0, 32, 64}: rows 32h..32h+16 = head h's [16, S]
        selexp3 = ph.tile([96, S], bf16, tag="selexp3", name="selexp3")
        # v-global rows replicated at the same offsets for the PV matmul
        vg3 = ph.tile([96, HD + 1], bf16, tag="vg3", name="vg3")
        eg = [
            ph.tile([128, NKC, G], bf16, tag=f"eg{h}", name=f"eg{h}")
            for h in range(HPC)
        ]
        outg = [ph.tile([G, HD], f32, tag=f"outg{h}", name=f"outg{h}") for h in range(HPC)]

        def mm(out, lhsT, rhs, start, stop):
            nc.tensor.matmul(out, lhsT, rhs, start=start, stop=stop)

        AFexp = AF.Exp

        def vall_slot_ap(ci, par, width=HD):
            # [128, h, d] AP over vall slots (par=0: v slots 0/2/4;
            # par=1: vg slots 1/3/5) of kpos chunk ci
            return bass.AP(
                tensor=vall.tensor,
                offset=vall.offset + (ci * 2 * HPC + par) * (HD + 1),
                ap=[vall.ap[0], [2 * (HD + 1), HPC], [1, width]],
            )

        # ---- projection s-tile body ----
        # part 0: q/k + kg + v/vg chunks 0-1 (everything band block 2s-1
        # needs); part 1: v/vg chunks 2-3 + sel (+ qg/vg3 at st 0). The
        # split lets band blocks slot between the halves so per-epoch
        # tensor utilization stays below the power governor's trip point.
        _xt_cache = {}

        def proj_stile(st, part):
            ssl = slice(512 * st, 512 * (st + 1))
            if part == 0:
                if st == 0:
                    _xt_cache[st] = (xt0, xt80)
                else:
                    _xt_cache[st] = (x_tiles(ssl, (3, 3)), x8_tile(ssl))
            xt, xt8 = _xt_cache[st]
            if part == 1:
                del _xt_cache[st]

            # q/k packed: transposed layout, W stationary, 3 full PSUM
            # tiles [q0;q1], [k0;k1], [q2;k2]
            for dc in range(3 if part == 0 else 0):
                d0 = 128 * dc
                ps = psB.tile([128, 512], f32, tag="small", name="psqk")
                for kc in range(6):
                    mm(ps, wqk[:, kc, d0 : d0 + 128], xt(kc), kc == 0, kc == 5)
                if dc == 0:
                    nc.vector.tensor_scalar_add(P0[:, ssl], ps, bias["qk"][:, 0:1])
                elif dc == 1:
                    nc.vector.tensor_scalar_add(P1[:, ssl], ps, bias["qk"][:, 1:2])
                else:
                    nc.vector.tensor_scalar_add(
                        q2[:, ssl], ps[0:64, :], bias["qk"][0:64, 2:3]
                    )
                    nc.vector.tensor_scalar_add(
                        k2[:, ssl], ps[64:128, :], bias["qk"][64:128, 2:3]
                    )

            # kg: fp8 DoubleRow, transposed layout, W stationary
            for ti, (d0, d1) in enumerate(
                ((0, 128), (128, 192)) if part == 0 else ()
            ):
                ps = psB.tile([d1 - d0, 512], f32, tag="small", name="pskg")
                for p in range(3):
                    nc.tensor.matmul(
                        ps,
                        w6["kg"][:, 2 * p : 2 * p + 2, d0:d1],
                        xt8[:, 2 * p : 2 * p + 2, :],
                        start=(p == 0),
                        stop=(p == 2),
                        perf_mode=DR,
                    )
                dst = KG01[:, ssl] if ti == 0 else kg2[:, ssl]
                nc.vector.tensor_scalar(
                    dst,
                    ps,
                    1.0 / FP8S,
                    bias["kg"][0 : d1 - d0, ti : ti + 1],
                    ALU.mult,
                    ALU.add,
                )

            # v: natural layout, xT chunks stationary (bf16)
            for sc in (range(0, 2) if part == 0 else range(2, 4)):
                ci = 4 * st + sc
                msl = slice(128 * sc, 128 * (sc + 1))
                psv = psB.tile([128, DPC], f32, tag="small", name="psv")
                for kc in range(6):
                    mm(psv, xt(kc, msl), w6["v"][:, kc, :], kc == 0, kc == 5)
                nc.vector.tensor_add(
                    vall_slot_ap(ci, 0),
                    psv[:, :].rearrange("p (h d) -> p h d", h=HPC),
                    bv_sb,
                )

                # vg: natural layout, fp8 DoubleRow, xT chunks stationary
                psg = psB.tile([128, DPC], f32, tag="small", name="psvg")
                for p in range(3):
                    nc.tensor.matmul(
                        psg,
                        xt8[:, 2 * p : 2 * p + 2, msl],
                        w6["vg"][:, 2 * p : 2 * p + 2, :],
                        start=(p == 0),
                        stop=(p == 2),
                        perf_mode=DR,
                    )
                nc.vector.scalar_tensor_tensor(
                    vall_slot_ap(ci, 1),
                    psg[:, :].rearrange("p (h d) -> p h d", h=HPC),
                    1.0 / FP8S,
                    bvg_sb,
                    ALU.mult,
                    ALU.add,
                )

            if part == 0:
                return

            # global columns for this s-tile: sel = q . k[:G], all heads
            # packed into one [96, 512] PSUM tile so the exp uses 96 lanes
            sps = psB.tile([96, 512], f32, tag="small", name="sps")
            for h in range(HPC):
                mm(
                    sps[32 * h : 32 * h + G, :],
                    kTh(h, slice(0, G)),
                    qTh(h, ssl),
                    True,
                    True,
                )
            nc.scalar.activation(out=selexp3[:, ssl], in_=sps, func=AFexp)

            if st == 0:
                # qg: heads 0/1 into one [128, G] PSUM tile, head 2 separate
                psq = psB.tile([128, G], f32, tag="small", name="psqg")
                for mq in range(2):
                    for kc in range(6):
                        mm(
                            psq[64 * mq : 64 * mq + 64, :],
                            w6["qg"][:, kc, 64 * mq : 64 * mq + 64],
                            xt(kc, slice(0, G)),
                            kc == 0,
                            kc == 5,
                        )
                nc.vector.tensor_scalar_add(QG01, psq, bias["qg"][:, 0:1])
                psq2 = psB.tile([64, G], f32, tag="small", name="psqg2")
                for kc in range(6):
                    mm(psq2, w6["qg"][:, kc, 128:192], xt(kc, slice(0, G)), kc == 0, kc == 5)
                nc.vector.tensor_scalar_add(qg2, psq2, bias["qg"][0:64, 1:2])
                # replicate v-global rows (chunk 0, slots 0/2/4, incl. ones
                # col) to partition offsets {0,32,64} for the sel-PV matmul
                for h in range(HPC):
                    nc.sync.dma_start(
                        out=vg3[32 * h : 32 * h + G, :], in_=vall[0:G, 0, 2 * h, :]
                    )

        # ---- banded local attention block ----
        # Each 128-query half only consumes 5 of the block's 6 kpos chunks,
        # so the two half-specific edge chunks (c=0 -> half 0 / c=5 ->
        # half 1) are computed at N=128 and share score slot 0.
        mask_rr = [0]

        # block 0's staging tile persists: its rows 0..15 are overwritten
        # with the global-row outputs at the very end
        osb3_0 = ph.tile([128, 2, HPC, HD], f32, tag="osb3_0", name="osb3_0")

        def band_block(t):
            # one output staging tile per block: [q mod 128, half, head, d];
            # a single batched DMA writes all 256 rows x 192 cols after the
            # three heads finish
            osb3 = osb3_0 if t == 0 else sbS.tile(
                [128, 2, HPC, HD], f32, tag="osb3", name="osb3"
            )
            cl, ch = _chunk_range(t)
            bexps = {}

            def qk_part(h):
                sc_ps = psA.tile([128, 5, 256], f32, tag="scores", name="sc_ps")
                for c in range(cl, ch):
                    j = 2 * t - 2 + c
                    if c == 0:
                        dst, qs = sc_ps[:, 0, 0:128], slice(256 * t, 256 * t + 128)
                    elif c == 5:
                        dst, qs = (
                            sc_ps[:, 0, 128:256],
                            slice(256 * t + 128, 256 * t + 256),
                        )
                    else:
                        dst, qs = sc_ps[:, c, :], slice(256 * t, 256 * (t + 1))
                    mm(dst, kTh(h, slice(128 * j, 128 * (j + 1))), qTh(h, qs), True, True)
                bexp = bx.tile([128, 5, 256], bf16, tag="bexp", name="bexp")
                nc.scalar.activation(out=bexp, in_=sc_ps, func=AFexp)
                for c in range(cl, ch):
                    for nm, half in mask_apply[(t, c)]:
                        if nm is None:
                            continue
                        if c == 0:
                            sl, cs = 0, slice(0, 128)
                        elif c == 5:
                            sl, cs = 0, slice(128, 256)
                        else:
                            sl, cs = c, slice(128 * half, 128 * (half + 1))
                        eng = nc.vector if mask_rr[0] % 2 else nc.gpsimd
                        mask_rr[0] += 1
                        eng.tensor_mul(
                            bexp[:, sl, cs], bexp[:, sl, cs], masks_sb[:, midx[nm], :]
                        )
                bexps[h] = bexp

            def pv_part(h):
                bexp = bexps[h]
                for half in range(2):
                    q0 = 256 * t + 128 * half
                    chunks = [
                        c
                        for c in range(cl, ch)
                        if (2 * t + half) - 2 <= 2 * t - 2 + c <= (2 * t + half) + 2
                    ]
                    at = psB.tile([128, HD + 1], f32, tag="small", name="at")
                    for ci_, c in enumerate(chunks):
                        j = 2 * t - 2 + c
                        if c == 0:
                            sl, cs = 0, slice(0, 128)
                        elif c == 5:
                            sl, cs = 0, slice(128, 256)
                        else:
                            sl, cs = c, slice(128 * half, 128 * (half + 1))
                        mm(at, bexp[:, sl, cs], vall[:, j, 2 * h, :], ci_ == 0, False)
                    mm(
                        at,
                        selexp3[32 * h : 32 * h + G, q0 : q0 + 128],
                        vg3[32 * h : 32 * h + G, :],
                        False,
                        True,
                    )
                    rec = sbS.tile([128, 1], f32, tag="rec", name="rec")
                    nc.vector.reciprocal(rec, at[:, HD : HD + 1])
                    nc.vector.tensor_scalar_mul(osb3[:, half, h, :], at[:, 0:HD], rec)

            # per-head QK -> exp/mask -> PV; latency hides across the
            # adjacent interleaved blocks and projection s-tiles
            for h in range(HPC):
                qk_part(h)
                pv_part(h)
            if t == 0:
                # rows 0..15 wait for the global-row outputs; ship the rest
                nc.sync.dma_start(
                    out=out_d[G:128, :], in_=osb3[G:128, 0, :, :]
                )
                nc.sync.dma_start(
                    out=out_d[128:256, :], in_=osb3[:, 1, :, :]
                )
            else:
                nc.sync.dma_start(
                    out=out_d[256 * t : 256 * (t + 1), :].rearrange(
                        "(f p) c -> p f c", p=128
                    ),
                    in_=osb3,
                )

        # ---- schedule: interleave band blocks with projection halves ----
        proj_stile(0, 0)
        proj_stile(0, 1)
        proj_stile(1, 0)
        band_block(0)
        band_block(1)
        proj_stile(1, 1)
        band_block(2)
        for s in range(2, 8):
            proj_stile(s, 0)
            band_block(2 * s - 1)
            proj_stile(s, 1)
            band_block(2 * s)

        # ---- global-token rows: full attention with qg/kg/vg ----
        for h in range(HPC):
            gps = psB.tile([128, NKC, G], f32, tag="small", name="gps")
            for c in range(NKC):
                mm(
                    gps[:, c, :],
                    kgh(h, slice(128 * c, 128 * (c + 1))),
                    qgh(h),
                    True,
                    True,
                )
            nc.scalar.activation(out=eg[h], in_=gps, func=AFexp)
            ops = psB.tile([G, HD + 1], f32, tag="small", name="ops")
            for c in range(NKC):
                mm(ops, eg[h][:, c, :], vall[:, c, 2 * h + 1, :], c == 0, c == NKC - 1)
            recg = sbS.tile([G, 1], f32, tag="recg", name="recg")
            nc.vector.reciprocal(recg, ops[:, HD : HD + 1])
            nc.vector.tensor_scalar_mul(outg[h], ops[:, 0:HD], recg)

        band_block(15)

        # rows 0..15 take the global-row outputs
        for h in range(HPC):
            nc.vector.tensor_copy(out=osb3_0[0:G, 0, h, :], in_=outg[h])
        nc.sync.dma_start(out=out_d[0:G, :], in_=osb3_0[0:G, 0, :, :])

    return nc


def _get_program():
    if "nc" not in _CACHE:
        nc = _build_program()
        nc.finalize()
        _CACHE["nc"] = nc
    return _CACHE["nc"]


def _prep_in_maps(hidden_states, Wq, bq, Wk, bk, Wv, bv, Wqg, bqg, Wkg, bkg, Wvg, bvg):
    hs = np.asarray(hidden_states, dtype=np.float32)
    f32 = np.float32
    bf = ml_dtypes.bfloat16
    f8 = ml_dtypes.float8_e4m3
    in_maps = []
    for c in range(NCORES):
        b = c // 4
        cols = slice(HD * 3 * (c % 4), HD * (3 * (c % 4) + 3))

        def hseg(M, h, scale=1.0):
            return np.asarray(M)[:, cols][:, HD * h : HD * (h + 1)] * scale

        def bseg(v, h, scale=1.0):
            return (np.asarray(v)[cols][HD * h : HD * (h + 1)] * scale).astype(f32)

        def bbast(v):
            # [192] -> broadcast [128, 3, 64]
            a = np.asarray(v)[cols].reshape(HPC, HD).astype(f32)
            return np.ascontiguousarray(np.broadcast_to(a[None], (128, HPC, HD)))

        xTc = np.ascontiguousarray(hs[b].T)
        wqk = np.concatenate(
            [
                hseg(Wq, 0, SCALE), hseg(Wq, 1, SCALE),
                hseg(Wk, 0), hseg(Wk, 1),
                hseg(Wq, 2, SCALE), hseg(Wk, 2),
            ],
            axis=1,
        )
        bqk = np.stack(
            [
                np.concatenate([bseg(bq, 0, SCALE), bseg(bq, 1, SCALE)]),
                np.concatenate([bseg(bk, 0), bseg(bk, 1)]),
                np.concatenate([bseg(bq, 2, SCALE), bseg(bk, 2)]),
            ],
            axis=1,
        )
        bkg2 = np.stack(
            [
                np.concatenate([bseg(bkg, 0), bseg(bkg, 1)]),
                np.concatenate([bseg(bkg, 2), np.zeros(HD, f32)]),
            ],
            axis=1,
        )
        bqg2 = np.stack(
            [
                np.concatenate([bseg(bqg, 0, SCALE), bseg(bqg, 1, SCALE)]),
                np.concatenate([bseg(bqg, 2, SCALE), np.zeros(HD, f32)]),
            ],
            axis=1,
        )
        in_maps.append(
            {
                "xT": xTc.astype(bf),
                "x8T": xTc.astype(f8),
                "Wqk": np.ascontiguousarray(wqk).astype(bf),
                "W8kg": np.ascontiguousarray(np.asarray(Wkg)[:, cols] * FP8S).astype(f8),
                "Wv": np.ascontiguousarray(np.asarray(Wv)[:, cols]).astype(bf),
                "W8vg": np.ascontiguousarray(np.asarray(Wvg)[:, cols] * FP8S).astype(f8),
                "Wqg": np.ascontiguousarray(np.asarray(Wqg)[:, cols] * SCALE).astype(bf),
                "b_qk": np.ascontiguousarray(bqk),
                "b_kg": np.ascontiguousarray(bkg2),
                "b_qg": np.ascontiguousarray(bqg2),
                "b_v": bbast(bv),
                "b_vg": bbast(bvg),
            }
        )
    return in_maps


def kernel(
    hidden_states,
    Wq,
    bq,
    Wk,
    bk,
    Wv,
    bv,
    Wqg,
    bqg,
    Wkg,
    bkg,
    Wvg,
    bvg,
    n_global,
):
    from concourse.bass_utils import run_bass_kernel_spmd

    assert int(n_global) == G
    nc = _get_program()
    in_maps = _prep_in_maps(
        hidden_states, Wq, bq, Wk, bk, Wv, bv, Wqg, bqg, Wkg, bkg, Wvg, bvg
    )
    res = run_bass_kernel_spmd(nc, in_maps, list(range(NCORES)))
    out = np.zeros((B, S, Dm), np.float32)
    for c in range(NCORES):
        b = c // 4
        cols = slice(HD * 3 * (c % 4), HD * (3 * (c % 4) + 3))
        out[b, :, cols] = res.results[c]["out"]
    return out


# revision 55
# speedup vs baseline: 1.0408x; 1.0164x over previous
"""Longformer self-attention Trainium2 kernel (8-core SPMD).

Sharding: core c handles batch b = c//4 and heads [3*(c%4), 3*(c%4)+3).
Each core receives pre-sliced/augmented inputs and computes [4096, 192]
(its 3 heads' output dims); the host reassembles [2, 4096, 768].

Device-side math per core (heads h in 0..3, all layouts chosen so no
on-device transposes are needed):
  - xT [768, 4096] = hidden[b].T; q-scale folded into Wq/Wqg on host.
  - q/k projections packed into one [768, 384] weight (column order
    q0,q1,k0,k1,q2,k2) so PSUM tiles are full 128 rows and evacuate
    with full-lane DVE ops; heads 0/1 of each projection live stacked
    in one [128, S] SBUF tile (head h at partition base 64*(h%2), so
    every per-head matmul has lhsT/rhs at matching partition bases).
  - kg/vg (only consumed by the 16 global-token rows, whose softmax
    averages over all 4096 keys) are computed in fp8e4m3 with the
    DoubleRow perf mode (2 contraction chunks per instruction = 2x
    fewer PE instructions). Weights are pre-scaled by 64 on the host to
    sit in the e4m3 normal range; the 1/64 descale is folded into the
    bias-add evacuation.
  - Band scores computed transposed: sT[kpos, q]. Each 128-query half
    consumes only 5 kpos chunks, so the half-specific edge chunks
    (c=0 -> half 0, c=5 -> half 1) are computed at N=128 and share
    score slot 0 of a [128, 5, 256] PSUM tile.
  - exp() without max subtraction (logits are O(0.3)); band-validity
    and global-exclusion masks are [128, 128] triangles applied
    multiplicatively after the exp, split across DVE and gpsimd.
  - Global columns (sel): the three heads' [16, S] score tiles are
    packed at partition offsets {0, 32, 64} of one [96, S] tensor via
    matmul tile positioning so the exp runs on 96 lanes instead of 16.
  - PV: attn[q, 0:64] and the softmax denominator (ones column of v)
    come out of one accumulated PSUM [128, 65]; normalize = reciprocal
    + mul.
  - Band block t only needs projection s-tiles <= ceil(t/2), so blocks
    2s-1 and 2s are interleaved right after s-tile s: the band's
    scalar/DVE-heavy pipeline fills the projection phase's DMA/evac
    stalls and smooths tensor-engine utilization (the HW power governor
    halves the PE clock when utilization stays pinned near 100%).
  - Global-token rows (0..15) use the qg/kg/vg projections with the
    same transposed-score trick; block 0 (whose rows 0..15 they
    overwrite) runs last.
"""

import sys

sys.path.insert(0, "/opt/trn_rl_repo")

import numpy as np
import ml_dtypes

B, S, Dm, H, WIN, G, HD = 2, 4096, 768, 12, 256, 16, 64
HPC = 3            # heads per core
NCORES = 8
DPC = HPC * HD     # 192 output dims per core
NB = S // WIN      # 16 query blocks
NKC = S // 128     # 32 kpos chunks of 128
SCALE = 1.0 / 8.0  # 1/sqrt(64)
FP8S = 64.0        # fp8 weight pre-scale (host) / descale (evacuation)
QS8 = 512.0        # fp8 pre-scale for q columns (Wq*SCALE has std 0.0025)
KS8 = 64.0         # fp8 pre-scale for k columns

_CACHE = {}


def _mask_classes():
    """Multiplicative {0,1} masks in transposed-score orientation
    [kpos_local p, q_local r (within a 128-query half)], applied to
    exp(scores). Keep (1.0) iff the slot is band-valid and not a global
    key; masked slots contribute exactly 0 to the reference softmax
    (exp(-inf) / exp(x - 10000) both underflow to 0).

    Each 128-query half i (q = 128i + r) consumes kpos chunks
    j = i-2 .. i+2. Only the edge chunks need masks: j = i-2 keeps
    p >= r (lower triangle), j = i+2 keeps p <= r; chunk j = 0
    additionally excludes the global keys (p >= G). Interior chunks are
    fully valid. Returns {name: [128, 128] mask}, plus a per-(t, c)
    application list [(name, half)] verified against the reference
    condition.
    """
    p = np.arange(128)[:, None]
    r = np.arange(128)[None, :]
    classes = {
        "lowT": (p >= r).astype(np.float32),
        "upT": (p <= r).astype(np.float32),
        "lowTg16": ((p >= r) & (p >= G)).astype(np.float32),
        "g16": (p >= G).astype(np.float32) * np.ones((128, 128), np.float32),
    }

    def ref_keep(t, c, half):
        # reference validity of chunk c's slots for query half (t, half)
        kpos = (2 * t - 2 + c) * 128 + p
        i = 256 * t + 128 * half + r
        return (np.abs(kpos - i) <= WIN) & (kpos >= 0) & (kpos < S) & (kpos >= G)

    # application list per (t, c): [(class_name or None, half), ...]
    apply = {}
    for t in range(NB):
        cl, ch = _chunk_range(t)
        for c in range(cl, ch):
            j = 2 * t - 2 + c
            ents = []
            for half in range(2):
                i = 2 * t + half
                if not (i - 2 <= j <= i + 2):
                    continue  # this half never consumes chunk c
                if j == i - 2:
                    nm = "lowTg16" if j == 0 else "lowT"
                elif j == i + 2:
                    nm = "upT"
                elif j == 0:
                    nm = "g16"
                else:
                    nm = None
                if nm is not None:
                    assert np.array_equal(
                        classes[nm].astype(bool), ref_keep(t, c, half)
                    ), (t, c, half, nm)
                else:
                    assert np.all(ref_keep(t, c, half)), (t, c, half)
                ents.append((nm, half))
            apply[(t, c)] = ents
    return classes, apply


def _chunk_range(t):
    if t == 0:
        return 2, 6
    if t == NB - 1:
        return 0, 4
    return 0, 6


def _patch_drain_and_barrier():
    """The walrus build in this container rejects >1 sync-wait on the CTRL
    (Drain) instruction that TileContext emits at exit ("Too many sync wait
    commands"). Split the waits: keep one on the drain, emit the rest as
    explicit single-sem wait_ge instructions on the sync engine before the
    barrier. Semantics preserved: all sems still quiesce before the
    sem-clear + barrier."""
    import concourse.tile as tile
    from concourse import mybir
    from concourse.vector_clock import ScopedClock

    if getattr(tile.TileContext, "_ant_drain_patch", False):
        return

    def _drain_and_barrier(self, tick_clock, wait_clock):
        nc = self.nc
        drain_inst = nc.sync.drain()
        wait_clock.add_sem_waits(
            drain_inst.ins, ScopedClock({None: tick_clock.global_clock})
        )
        si = drain_inst.ins.sync_info
        waits = list(si.on_wait) if si is not None else []
        if len(waits) > 1:
            drain_inst.ins.sync_info = mybir.SyncInfo(
                on_wait=[waits[0]], on_update=list(si.on_update)
            )
            allocated = self.sems.allocated()
            by_name = {}
            for key, sem in allocated.items():
                by_name[str(key)] = sem
                nm = getattr(sem, "name", None)
                if nm is not None:
                    by_name[str(nm)] = sem
            for w in waits[1:]:
                sem = by_name[w.ant_name]
                nc.sync.wait_ge(sem, w.wait_value)
        nc.all_engine_barrier()
        assert self.sems is not None
        popped = nc._tile_sem_poison_stack.pop()
        assert popped is self._sem_poison
        nc.clear_and_free_semaphores(list(self.sems.allocated().values()))
        nc.all_engine_barrier()

    tile.TileContext._drain_and_barrier = _drain_and_barrier
    tile.TileContext._ant_drain_patch = True


def _build_program():
    import concourse.bass as bass
    import concourse.tile as tile
    from concourse import bacc, mybir

    _patch_drain_and_barrier()

    f32 = mybir.dt.float32
    bf16 = mybir.dt.bfloat16
    fp8 = mybir.dt.float8e4
    AF = mybir.ActivationFunctionType
    ALU = mybir.AluOpType
    DR = mybir.MatmulPerfMode.DoubleRow

    # Bacc (not plain Bass): its compile() pipeline runs
    # generate_event_semaphores, which splits multi-sem waits — this
    # walrus build allows at most one sync wait per instruction.
    nc = bacc.Bacc(None)

    xT = nc.dram_tensor("xT", [Dm, S], bf16, kind="ExternalInput")
    x8T = nc.dram_tensor("x8T", [Dm, S], fp8, kind="ExternalInput")
    # column order q0,q1,k0,k1,q2,k2 (64 cols each; q cols pre-scaled)
    Wqk = nc.dram_tensor("Wqk", [Dm, 2 * DPC], bf16, kind="ExternalInput")
    W8kg = nc.dram_tensor("W8kg", [Dm, DPC], fp8, kind="ExternalInput")
    Wv = nc.dram_tensor("Wv", [Dm, DPC], bf16, kind="ExternalInput")
    W8vg = nc.dram_tensor("W8vg", [Dm, DPC], fp8, kind="ExternalInput")
    Wqg = nc.dram_tensor("Wqg", [Dm, DPC], bf16, kind="ExternalInput")
    # stacked bias columns: col layout matches the packed PSUM tiles
    b_qk = nc.dram_tensor("b_qk", [128, 3], f32, kind="ExternalInput")
    b_kg = nc.dram_tensor("b_kg", [128, 2], f32, kind="ExternalInput")
    b_qg = nc.dram_tensor("b_qg", [128, 2], f32, kind="ExternalInput")
    # broadcast v/vg biases: [128 partitions, head, 64]
    b_v = nc.dram_tensor("b_v", [128, HPC, HD], f32, kind="ExternalInput")
    b_vg = nc.dram_tensor("b_vg", [128, HPC, HD], f32, kind="ExternalInput")
    out_d = nc.dram_tensor("out", [S, DPC], f32, kind="ExternalOutput")

    classes, mask_apply = _mask_classes()
    mask_names = list(classes.keys())
    mask_np = np.stack([classes[k] for k in mask_names], axis=1)  # [128, 4, 128]
    masks_d = nc.inline_tensor(mask_np.astype(ml_dtypes.bfloat16), name="masks")
    midx = {k: i for i, k in enumerate(mask_names)}

    from contextlib import ExitStack

    with tile.TileContext(nc) as tc, ExitStack() as ctx:
        const = ctx.enter_context(tc.tile_pool(name="const", bufs=1))
        ph = ctx.enter_context(tc.tile_pool(name="ph", bufs=1))
        xpool = ctx.enter_context(tc.tile_pool(name="xpool", bufs=14))
        x8pool = ctx.enter_context(tc.tile_pool(name="x8pool", bufs=3))
        bx = ctx.enter_context(tc.tile_pool(name="bx", bufs=4))
        sbS = ctx.enter_context(tc.tile_pool(name="sbS", bufs=6))
        psA = ctx.enter_context(tc.tile_pool(name="psA", bufs=2, space="PSUM"))
        psB = ctx.enter_context(tc.tile_pool(name="psB", bufs=2, space="PSUM"))

        # issue the first projection group's operands first (Wqk chunk 0,
        # x chunk 0) so the PE starts within ~1us of kernel entry
        wqk = const.tile([128, 6, 2 * DPC], bf16, tag="wqk", name="wqk")
        nc.sync.dma_start(out=wqk[:, 0, :], in_=Wqk[0:128, :])
        nc.sync.dma_start(out=wqk[:, 1, :], in_=Wqk[128:256, :])

        def x_tiles(ssl, splits):
            # bf16 x chunks as len(splits) tiles; returns accessor kc -> AP
            tiles, offs = [], []
            k0 = 0
            for n in splits:
                t = xpool.tile([128, n, 512], bf16, tag="xt", name="xt")
                nc.sync.dma_start(
                    out=t,
                    in_=xT[128 * k0 : 128 * (k0 + n), ssl].rearrange(
                        "(c p) s -> p c s", p=128
                    ),
                )
                tiles.append(t)
                offs.append(k0)
                k0 += n
            def xtc(kc, cs=slice(0, 512)):
                for t, o, n in zip(tiles, offs, splits):
                    if o <= kc < o + n:
                        return t[:, kc - o, cs]
                raise KeyError(kc)
            return xtc

        def x8_tile(ssl):
            t8 = x8pool.tile([128, 6, 512], fp8, tag="xt8", name="xt8")
            nc.sync.dma_start(
                out=t8, in_=x8T[:, ssl].rearrange("(c p) s -> p c s", p=128)
            )
            return t8

        xt0 = x_tiles(slice(0, 512), (1, 1, 2, 2))
        nc.sync.dma_start(
            out=wqk[:, 2:6, :],
            in_=Wqk[256:768, :].rearrange("(c p) d -> p c d", p=128),
        )
        xt80 = x8_tile(slice(0, 512))

        # ---- remaining constants to SBUF ----
        w6 = {}
        for nm, dram, width, dt in (
            ("kg", W8kg, DPC, fp8),
            ("v", Wv, DPC, bf16),
            ("vg", W8vg, DPC, fp8),
            ("qg", Wqg, DPC, bf16),
        ):
            w6[nm] = const.tile([128, 6, width], dt, tag=f"w6{nm}", name=f"w6{nm}")
            nc.sync.dma_start(
                out=w6[nm], in_=dram[:, :].rearrange("(c p) d -> p c d", p=128)
            )
        bias = {}
        for nm, dram, w in (("qk", b_qk, 3), ("kg", b_kg, 2), ("qg", b_qg, 2)):
            bias[nm] = const.tile([128, w], f32, tag=f"b{nm}", name=f"b{nm}")
            nc.sync.dma_start(out=bias[nm], in_=dram[:])
        bv_sb = const.tile([128, HPC, HD], f32, tag="bv", name="bv_sb")
        nc.sync.dma_start(out=bv_sb, in_=b_v[:])
        bvg_sb = const.tile([128, HPC, HD], f32, tag="bvg", name="bvg_sb")
        nc.sync.dma_start(out=bvg_sb, in_=b_vg[:])
        masks_sb = const.tile([128, 4, 128], bf16, tag="masks", name="masks_sb")
        nc.sync.dma_start(out=masks_sb, in_=masks_d[:])

        # ---- persistent per-head tensors (heads 0/1 stacked per tile) ----
        P0 = ph.tile([128, S], bf16, tag="P0", name="P0")   # [q0; q1]
        P1 = ph.tile([128, S], bf16, tag="P1", name="P1")   # [k0; k1]
        q2 = ph.tile([64, S], bf16, tag="q2", name="q2")
        k2 = ph.tile([64, S], bf16, tag="k2", name="k2")
        KG01 = ph.tile([128, S], bf16, tag="KG01", name="KG01")
        kg2 = ph.tile([64, S], bf16, tag="kg2", name="kg2")
        QG01 = ph.tile([128, G], bf16, tag="QG01", name="QG01")
        qg2 = ph.tile([64, G], bf16, tag="qg2", name="qg2")

        def qTh(h, cs):
            return P0[64 * h : 64 * h + 64, cs] if h < 2 else q2[:, cs]

        def kTh(h, cs):
            return P1[64 * h : 64 * h + 64, cs] if h < 2 else k2[:, cs]

        def kgh(h, cs):
            return KG01[64 * h : 64 * h + 64, cs] if h < 2 else kg2[:, cs]

        def qgh(h):
            return QG01[64 * h : 64 * h + 64, :] if h < 2 else qg2[:, :]

        # v/vg interleaved with ones column: [:, chunk, 2h+0, :] = v head h,
        # [:, chunk, 2h+1, :] = vg head h ([:, :, :, 64] = 1.0)
        vall = ph.tile([128, NKC, 2 * HPC, HD + 1], bf16, tag="vall", name="vall")
        nc.vector.memset(vall[:, :, :, HD : HD + 1], 1.0)
        # three heads' global-column exp'd scores packed at partition
        # offsets {0, 32, 64}: rows 32h..32h+16 = head h's [16, S]
        selexp3 = ph.tile([96, S], bf16, tag="selexp3", name="selexp3")
        # v-global rows replicated at the same offsets for the PV matmul
        vg3 = ph.tile([96, HD + 1], bf16, tag="vg3", name="vg3")
        eg = [
            ph.tile([128, NKC, G], bf16, tag=f"eg{h}", name=f"eg{h}")
            for h in range(HPC)
        ]
        outg = [ph.tile([G, HD], f32, tag=f"outg{h}", name=f"outg{h}") for h in range(HPC)]

        def mm(out, lhsT, rhs, start, stop):
            nc.tensor.matmul(out, lhsT, rhs, start=start, stop=stop)

        AFexp = AF.Exp

        def vall_slot_ap(ci, par, width=HD):
            # [128, h, d] AP over vall slots (par=0: v slots 0/2/4;
            # par=1: vg slots 1/3/5) of kpos chunk ci
            return bass.AP(
                tensor=vall.tensor,
                offset=vall.offset + (ci * 2 * HPC + par) * (HD + 1),
                ap=[vall.ap[0], [2 * (HD + 1), HPC], [1, width]],
            )

        # ---- projection s-tile body ----
        # part 0: q/k + kg + v/vg chunks 0-1 (everything band block 2s-1
        # needs); part 1: v/vg chunks 2-3 + sel (+ qg/vg3 at st 0). The
        # split lets band blocks slot between the halves so per-epoch
        # tensor utilization stays below the power governor's trip point.
        _xt_cache = {}

        def proj_stile(st, part):
            ssl = slice(512 * st, 512 * (st + 1))
            if part == 0:
                if st == 0:
                    _xt_cache[st] = (xt0, xt80)
                else:
                    _xt_cache[st] = (x_tiles(ssl, (3, 3)), x8_tile(ssl))
            xt, xt8 = _xt_cache[st]
            if part == 1:
                del _xt_cache[st]

            # q/k packed: transposed layout, W stationary, 3 full PSUM
            # tiles [q0;q1], [k0;k1], [q2;k2]
            for dc in range(3 if part == 0 else 0):
                d0 = 128 * dc
                ps = psB.tile([128, 512], f32, tag="small", name="psqk")
                for kc in range(6):
                    mm(ps, wqk[:, kc, d0 : d0 + 128], xt(kc), kc == 0, kc == 5)
                if dc == 0:
                    nc.vector.tensor_scalar_add(P0[:, ssl], ps, bias["qk"][:, 0:1])
                elif dc == 1:
                    nc.vector.tensor_scalar_add(P1[:, ssl], ps, bias["qk"][:, 1:2])
                else:
                    nc.vector.tensor_scalar_add(
                        q2[:, ssl], ps[0:64, :], bias["qk"][0:64, 2:3]
                    )
                    nc.vector.tensor_scalar_add(
                        k2[:, ssl], ps[64:128, :], bias["qk"][64:128, 2:3]
                    )

            # kg: fp8 DoubleRow, transposed layout, W stationary
            for ti, (d0, d1) in enumerate(
                ((0, 128), (128, 192)) if part == 0 else ()
            ):
                ps = psB.tile([d1 - d0, 512], f32, tag="small", name="pskg")
                for p in range(3):
                    nc.tensor.matmul(
                        ps,
                        w6["kg"][:, 2 * p : 2 * p + 2, d0:d1],
                        xt8[:, 2 * p : 2 * p + 2, :],
                        start=(p == 0),
                        stop=(p == 2),
                        perf_mode=DR,
                    )
                dst = KG01[:, ssl] if ti == 0 else kg2[:, ssl]
                nc.vector.tensor_scalar(
                    dst,
                    ps,
                    1.0 / FP8S,
                    bias["kg"][0 : d1 - d0, ti : ti + 1],
                    ALU.mult,
                    ALU.add,
                )

            # v: natural layout, xT chunks stationary (bf16)
            for sc in (range(0, 2) if part == 0 else range(2, 4)):
                ci = 4 * st + sc
                msl = slice(128 * sc, 128 * (sc + 1))
                psv = psB.tile([128, DPC], f32, tag="small", name="psv")
                for kc in range(6):
                    mm(psv, xt(kc, msl), w6["v"][:, kc, :], kc == 0, kc == 5)
                nc.vector.tensor_add(
                    vall_slot_ap(ci, 0),
                    psv[:, :].rearrange("p (h d) -> p h d", h=HPC),
                    bv_sb,
                )

                # vg: natural layout, fp8 DoubleRow, xT chunks stationary
                psg = psB.tile([128, DPC], f32, tag="small", name="psvg")
                for p in range(3):
                    nc.tensor.matmul(
                        psg,
                        xt8[:, 2 * p : 2 * p + 2, msl],
                        w6["vg"][:, 2 * p : 2 * p + 2, :],
                        start=(p == 0),
                        stop=(p == 2),
                        perf_mode=DR,
                    )
                nc.vector.scalar_tensor_tensor(
                    vall_slot_ap(ci, 1),
                    psg[:, :].rearrange("p (h d) -> p h d", h=HPC),
                    1.0 / FP8S,
                    bvg_sb,
                    ALU.mult,
                    ALU.add,
                )

            if part == 0:
                return

            # global columns for this s-tile: sel = q . k[:G], all heads
            # packed into one [96, 512] PSUM tile so the exp uses 96 lanes
            sps = psB.tile([96, 512], f32, tag="small", name="sps")
            for h in range(HPC):
                mm(
                    sps[32 * h : 32 * h + G, :],
                    kTh(h, slice(0, G)),
                    qTh(h, ssl),
                    True,
                    True,
                )
            nc.scalar.activation(out=selexp3[:, ssl], in_=sps, func=AFexp)

            if st == 0:
                # qg: heads 0/1 into one [128, G] PSUM tile, head 2 separate
                psq = psB.tile([128, G], f32, tag="small", name="psqg")
                for mq in range(2):
                    for kc in range(6):
                        mm(
                            psq[64 * mq : 64 * mq + 64, :],
                            w6["qg"][:, kc, 64 * mq : 64 * mq + 64],
                            xt(kc, slice(0, G)),
                            kc == 0,
                            kc == 5,
                        )
                nc.vector.tensor_scalar_add(QG01, psq, bias["qg"][:, 0:1])
                psq2 = psB.tile([64, G], f32, tag="small", name="psqg2")
                for kc in range(6):
                    mm(psq2, w6["qg"][:, kc, 128:192], xt(kc, slice(0, G)), kc == 0, kc == 5)
                nc.vector.tensor_scalar_add(qg2, psq2, bias["qg"][0:64, 1:2])
                # replicate v-global rows (chunk 0, slots 0/2/4, incl. ones
                # col) to partition offsets {0,32,64} for the sel-PV matmul
                for h in range(HPC):
                    nc.sync.dma_start(
                        out=vg3[32 * h : 32 * h + G, :], in_=vall[0:G, 0, 2 * h, :]
                    )

        # ---- banded local attention block ----
        # Each 128-query half only consumes 5 of the block's 6 kpos chunks,
        # so the two half-specific edge chunks (c=0 -> half 0 / c=5 ->
        # half 1) are computed at N=128 and share score slot 0.
        mask_rr = [0]

        # block 0's staging tile persists: its rows 0..15 are overwritten
        # with the global-row outputs at the very end
        osb3_0 = ph.tile([128, 2, HPC, HD], f32, tag="osb3_0", name="osb3_0")

        def band_block(t):
            # one output staging tile per block: [q mod 128, half, head, d];
            # a single batched DMA writes all 256 rows x 192 cols after the
            # three heads finish
            osb3 = osb3_0 if t == 0 else sbS.tile(
                [128, 2, HPC, HD], f32, tag="osb3", name="osb3"
            )
            cl, ch = _chunk_range(t)
            bexps = {}

            def qk_part(h):
                sc_ps = psA.tile([128, 5, 256], f32, tag="scores", name="sc_ps")

                def do_mm(c):
                    j = 2 * t - 2 + c
                    if c == 0:
                        dst, qs = sc_ps[:, 0, 0:128], slice(256 * t, 256 * t + 128)
                    elif c == 5:
                        dst, qs = (
                            sc_ps[:, 0, 128:256],
                            slice(256 * t + 128, 256 * t + 256),
                        )
                    else:
                        dst, qs = sc_ps[:, c, :], slice(256 * t, 256 * (t + 1))
                    mm(dst, kTh(h, slice(128 * j, 128 * (j + 1))), qTh(h, qs), True, True)

                def do_masks(bexp, cset):
                    for c in cset:
                        for nm, half in mask_apply[(t, c)]:
                            if nm is None:
                                continue
                            if c == 0:
                                sl, cs = 0, slice(0, 128)
                            elif c == 5:
                                sl, cs = 0, slice(128, 256)
                            else:
                                sl, cs = c, slice(128 * half, 128 * (half + 1))
                            eng = nc.vector if mask_rr[0] % 2 else nc.gpsimd
                            mask_rr[0] += 1
                            eng.tensor_mul(
                                bexp[:, sl, cs], bexp[:, sl, cs],
                                masks_sb[:, midx[nm], :],
                            )

                # slots 0-1 first, exp'd while slots 2-4 compute: PV's first
                # chunks unblock half an exp earlier
                grpA = [c for c in (0, 5, 1) if cl <= c < ch]
                grpB = [c for c in (2, 3, 4) if cl <= c < ch]
                bexp = bx.tile([128, 5, 256], bf16, tag="bexp", name="bexp")
                for c in grpA:
                    do_mm(c)
                nc.scalar.activation(
                    out=bexp[:, 0:2, :], in_=sc_ps[:, 0:2, :], func=AFexp
                )
                for c in grpB:
                    do_mm(c)
                do_masks(bexp, grpA)
                nc.scalar.activation(
                    out=bexp[:, 2:5, :], in_=sc_ps[:, 2:5, :], func=AFexp
                )
                do_masks(bexp, grpB)
                bexps[h] = bexp

            def pv_part(h):
                bexp = bexps[h]
                for half in range(2):
                    q0 = 256 * t + 128 * half
                    chunks = [
                        c
                        for c in range(cl, ch)
                        if (2 * t + half) - 2 <= 2 * t - 2 + c <= (2 * t + half) + 2
                    ]
                    at = psB.tile([128, HD + 1], f32, tag="small", name="at")
                    for ci_, c in enumerate(chunks):
                        j = 2 * t - 2 + c
                        if c == 0:
                            sl, cs = 0, slice(0, 128)
                        elif c == 5:
                            sl, cs = 0, slice(128, 256)
                        else:
                            sl, cs = c, slice(128 * half, 128 * (half + 1))
                        mm(at, bexp[:, sl, cs], vall[:, j, 2 * h, :], ci_ == 0, False)
                    mm(
                        at,
                        selexp3[32 * h : 32 * h + G, q0 : q0 + 128],
                        vg3[32 * h : 32 * h + G, :],
                        False,
                        True,
                    )
                    rec = sbS.tile([128, 1], f32, tag="rec", name="rec")
                    nc.vector.reciprocal(rec, at[:, HD : HD + 1])
                    nc.vector.tensor_scalar_mul(osb3[:, half, h, :], at[:, 0:HD], rec)

            # per-head QK -> exp/mask -> PV; latency hides across the
            # adjacent interleaved blocks and projection s-tiles
            for h in range(HPC):
                qk_part(h)
                pv_part(h)
            if t == 0:
                # rows 0..15 wait for the global-row outputs; ship the rest
                nc.sync.dma_start(
                    out=out_d[G:128, :], in_=osb3[G:128, 0, :, :]
                )
                nc.sync.dma_start(
                    out=out_d[128:256, :], in_=osb3[:, 1, :, :]
                )
            else:
                nc.sync.dma_start(
                    out=out_d[256 * t : 256 * (t + 1), :].rearrange(
                        "(f p) c -> p f c", p=128
                    ),
                    in_=osb3,
                )

        # ---- schedule: interleave band blocks with projection halves ----
        proj_stile(0, 0)
        proj_stile(0, 1)
        proj_stile(1, 0)
        band_block(0)
        band_block(1)
        proj_stile(1, 1)
        band_block(2)
        for s in range(2, 8):
            proj_stile(s, 0)
            band_block(2 * s - 1)
            proj_stile(s, 1)
            band_block(2 * s)

        # ---- global-token rows: full attention with qg/kg/vg ----
        for h in range(HPC):
            gps = psB.tile([128, NKC, G], f32, tag="small", name="gps")
            for c in range(NKC):
                mm(
                    gps[:, c, :],
                    kgh(h, slice(128 * c, 128 * (c + 1))),
                    qgh(h),
                    True,
                    True,
                )
            nc.scalar.activation(out=eg[h], in_=gps, func=AFexp)
            ops = psB.tile([G, HD + 1], f32, tag="small", name="ops")
            for c in range(NKC):
                mm(ops, eg[h][:, c, :], vall[:, c, 2 * h + 1, :], c == 0, c == NKC - 1)
            recg = sbS.tile([G, 1], f32, tag="recg", name="recg")
            nc.vector.reciprocal(recg, ops[:, HD : HD + 1])
            nc.vector.tensor_scalar_mul(outg[h], ops[:, 0:HD], recg)

        band_block(15)

        # rows 0..15 take the global-row outputs
        for h in range(HPC):
            nc.vector.tensor_copy(out=osb3_0[0:G, 0, h, :], in_=outg[h])
        nc.sync.dma_start(out=out_d[0:G, :], in_=osb3_0[0:G, 0, :, :])

    return nc


def _get_program():
    if "nc" not in _CACHE:
        nc = _build_program()
        nc.finalize()
        _CACHE["nc"] = nc
    return _CACHE["nc"]


def _prep_in_maps(hidden_states, Wq, bq, Wk, bk, Wv, bv, Wqg, bqg, Wkg, bkg, Wvg, bvg):
    hs = np.asarray(hidden_states, dtype=np.float32)
    f32 = np.float32
    bf = ml_dtypes.bfloat16
    f8 = ml_dtypes.float8_e4m3
    in_maps = []
    for c in range(NCORES):
        b = c // 4
        cols = slice(HD * 3 * (c % 4), HD * (3 * (c % 4) + 3))

        def hseg(M, h, scale=1.0):
            return np.asarray(M)[:, cols][:, HD * h : HD * (h + 1)] * scale

        def bseg(v, h, scale=1.0):
            return (np.asarray(v)[cols][HD * h : HD * (h + 1)] * scale).astype(f32)

        def bbast(v):
            # [192] -> broadcast [128, 3, 64]
            a = np.asarray(v)[cols].reshape(HPC, HD).astype(f32)
            return np.ascontiguousarray(np.broadcast_to(a[None], (128, HPC, HD)))

        xTc = np.ascontiguousarray(hs[b].T)
        wqk = np.concatenate(
            [
                hseg(Wq, 0, SCALE), hseg(Wq, 1, SCALE),
                hseg(Wk, 0), hseg(Wk, 1),
                hseg(Wq, 2, SCALE), hseg(Wk, 2),
            ],
            axis=1,
        )
        bqk = np.stack(
            [
                np.concatenate([bseg(bq, 0, SCALE), bseg(bq, 1, SCALE)]),
                np.concatenate([bseg(bk, 0), bseg(bk, 1)]),
                np.concatenate([bseg(bq, 2, SCALE), bseg(bk, 2)]),
            ],
            axis=1,
        )
        bkg2 = np.stack(
            [
                np.concatenate([bseg(bkg, 0), bseg(bkg, 1)]),
                np.concatenate([bseg(bkg, 2), np.zeros(HD, f32)]),
            ],
            axis=1,
        )
        bqg2 = np.stack(
            [
                np.concatenate([bseg(bqg, 0, SCALE), bseg(bqg, 1, SCALE)]),
                np.concatenate([bseg(bqg, 2, SCALE), np.zeros(HD, f32)]),
            ],
            axis=1,
        )
        in_maps.append(
            {
                "xT": xTc.astype(bf),
                "x8T": xTc.astype(f8),
                "Wqk": np.ascontiguousarray(wqk).astype(bf),
                "W8kg": np.ascontiguousarray(np.asarray(Wkg)[:, cols] * FP8S).astype(f8),
                "Wv": np.ascontiguousarray(np.asarray(Wv)[:, cols]).astype(bf),
                "W8vg": np.ascontiguousarray(np.asarray(Wvg)[:, cols] * FP8S).astype(f8),
                "Wqg": np.ascontiguousarray(np.asarray(Wqg)[:, cols] * SCALE).astype(bf),
                "b_qk": np.ascontiguousarray(bqk),
                "b_kg": np.ascontiguousarray(bkg2),
                "b_qg": np.ascontiguousarray(bqg2),
                "b_v": bbast(bv),
                "b_vg": bbast(bvg),
            }
        )
    return in_maps


def kernel(
    hidden_states,
    Wq,
    bq,
    Wk,
    bk,
    Wv,
    bv,
    Wqg,
    bqg,
    Wkg,
    bkg,
    Wvg,
    bvg,
    n_global,
):
    from concourse.bass_utils import run_bass_kernel_spmd

    assert int(n_global) == G
    nc = _get_program()
    in_maps = _prep_in_maps(
        hidden_states, Wq, bq, Wk, bk, Wv, bv, Wqg, bqg, Wkg, bkg, Wvg, bvg
    )
    res = run_bass_kernel_spmd(nc, in_maps, list(range(NCORES)))
    out = np.zeros((B, S, Dm), np.float32)
    for c in range(NCORES):
        b = c // 4
        cols = slice(HD * 3 * (c % 4), HD * (3 * (c % 4) + 3))
        out[b, :, cols] = res.results[c]["out"]
    return out
